# revision 38
# baseline (speedup 1.0000x reference)
"""Trainium2 Bass kernel for nn_EssentialMatrixEstimator (v3).

Distribution (8 cores):
  - XN: natural row-shard  (384 rows x 3072 cols) -> exact row top-3 thresholds.
  - XC: transposed col-shard (384 cols x 3072 rows as [col, row]) -> exact col
    top-3 thresholds + dense masking + col-sharded gram.
  - warmup collective (1B AllGather) issued first so the NRT entry barrier +
    cc-stream init overlap the input load phase.
  - coll1: AllGather of per-core row thresholds (384 f32 -> 3072).
  - coll2: AllGather of the per-core corrected 6x6 gram (vs AllReduce: lower
    floor); summed locally.

Math: the (N*M,9) epipolar Gram collapses to the 6x6 monomial Gram C'.
Monomials are pre-centered about the host constant c0 (grid centroid), so C'
is well-conditioned; the Hartley normalization is recovered from C' moments
(row/col 5) and applied as a 6x6 L-transform C2 = L1 C' L2^T instead of a
second gram pass.  Mmat (9x9) is an index expansion of C2; min-eigvector via
50-step shifted power iteration (rescaled repeated squaring), projection via
a 32-step 6x6 blockdiag chain.

v3 vs v2: candidate monomials computed arithmetically from gathered indices
(no m1tab indirect DMAs -> gpsimd free for the 9 post-AG threshold gathers);
correction products fused into one reduce; tail avoids the stage bounce for
T12/moments (PE row-extract + direct build from the broadcast strip); most
tail matmuls run f32r single-pass; per-queue load split (sync=XN, scalar=
consts+XN+XC, gpsimd=XC) so the threshold AllGather triggers early.
"""

import os

os.environ.setdefault("JAX_PLATFORMS", "axon")

import numpy as np

import concourse.bass as bass
import concourse.bass_isa as bass_isa
import concourse.mybir as mybir
import concourse.bacc as bacc
import concourse.tile as tile

NCORES = 8
N = 3072
SH = N // NCORES          # 384 rows/cols per core
RT = SH // 128            # 3 tiles per core shard
CB = N // 128             # 24 tiles across the full dim
F32 = mybir.dt.float32
F32R = mybir.dt.float32r
U32 = mybir.dt.uint32
U8 = mybir.dt.uint8
AF = mybir.ActivationFunctionType
OP = mybir.AluOpType
AX = mybir.AxisListType

EPS = 1e-8
SQRT2 = 1.4142135623730951
INV_SQRT3 = 1.0 / 1.7320508075688772
T0 = float(np.nextafter(np.float32(0.01), np.float32(1)))  # x > 0.01 == x >= T0
H, W = 64, 64

# cpack const layout (tensor [9, C_TOT]): column ranges
C_I9H = 0      # I9 * 0.5            [9, 9]
C_ET69 = 9     # E^T selector        [6, 9]
C_I3 = 18      # I3                  [3, 3]
C_V09 = 21     # full(1/3)           [9, 1]
C_V06 = 22     # full(1/sqrt3)       [6, 1]
C_SEL1 = 23    # [I3 | 0]            [3, 6]
C_SEL2 = 29    # [0 | I3]            [3, 6]
C_SHT = 35     # Sh component mats^T: I6, E1^T..E5^T   [6, 6*6]
C_MSK = 71     # svec masks [c2m c1m c0m]  [6, 3]
C_IDN = 74     # identity 9x9        [9, 9]
C_ONE = 83     # all-ones            [9, 9]
C_X5 = 92      # [0,0,1,0,0] row 0   [1, 5]  (rank-1 row-extract bases)
C_S3 = 97      # sel {0,4,8} cols    [9, 3]  (trace-of-Mmat selector)
C_M9 = 100     # mask 1@{0,4,8}      [3, 9]
C_TOT = 109

PAIRS = [(0, 0), (0, 1), (0, 2), (1, 1), (1, 2), (2, 2)]


def _pidx():
    d = {}
    for i, (a, b) in enumerate(PAIRS):
        d[(a, b)] = i
        d[(b, a)] = i
    return d


def grid_pts(K):
    idx = np.arange(H * W, dtype=np.float32)
    pix = np.stack([idx % np.float32(W), np.floor(idx / np.float32(W))], -1)
    K_inv = np.linalg.inv(np.asarray(K, np.float32)).astype(np.float32)
    p1h = np.concatenate([pix[:N], np.ones((N, 1), np.float32)], -1)
    pts = (p1h @ K_inv.T)[:, :2].astype(np.float32)
    return pts


def host_constants(K):
    """Pre-centered monomials + packed tail constants (f32)."""
    K = np.asarray(K, np.float32)
    pts = grid_pts(K)
    x, y = pts[:, 0], pts[:, 1]
    c0x = np.float32(x.mean())
    c0y = np.float32(y.mean())
    xs = (x - c0x).astype(np.float32)
    ys = (y - c0y).astype(np.float32)
    Mp = np.stack([xs * xs, xs * ys, xs, ys * ys, ys, np.ones_like(xs)],
                  -1).astype(np.float32)

    cpack = np.zeros((9, C_TOT), np.float32)
    cpack[:9, C_I9H:C_I9H + 9] = 0.5 * np.eye(9, dtype=np.float32)
    pid = _pidx()
    for a in range(3):
        for b in range(3):
            cpack[pid[(a, b)], C_ET69 + 3 * a + b] = 1.0
    cpack[:3, C_I3:C_I3 + 3] = np.eye(3, dtype=np.float32)
    cpack[:9, C_V09] = 1.0 / 3.0
    cpack[:6, C_V06] = INV_SQRT3
    cpack[:3, C_SEL1:C_SEL1 + 3] = np.eye(3, dtype=np.float32)
    cpack[:3, C_SEL2 + 3:C_SEL2 + 6] = np.eye(3, dtype=np.float32)

    # Sh(dx,dy) = I + dx*E1 + dy*E2 + dx^2*E3 + dx*dy*E4 + dy^2*E5
    E1 = np.zeros((6, 6), np.float32)  # dx terms
    E1[0, 2] = -2.0
    E1[1, 4] = -1.0
    E1[2, 5] = -1.0
    E2 = np.zeros((6, 6), np.float32)  # dy terms
    E2[1, 2] = -1.0
    E2[3, 4] = -2.0
    E2[4, 5] = -1.0
    E3 = np.zeros((6, 6), np.float32)  # dx^2
    E3[0, 5] = 1.0
    E4 = np.zeros((6, 6), np.float32)  # dx*dy
    E4[1, 5] = 1.0
    E5 = np.zeros((6, 6), np.float32)  # dy^2
    E5[3, 5] = 1.0
    mats = [np.eye(6, dtype=np.float32), E1, E2, E3, E4, E5]
    for i, Em in enumerate(mats):
        cpack[:6, C_SHT + 6 * i:C_SHT + 6 * i + 6] = Em.T
    # svec masks: svec = [s2,s2,s,s2,s,1] = c2m*s2 + c1m*s + c0m
    cpack[:6, C_MSK + 0] = [1, 1, 0, 1, 0, 0]
    cpack[:6, C_MSK + 1] = [0, 0, 1, 0, 1, 0]
    cpack[:6, C_MSK + 2] = [0, 0, 0, 0, 0, 1]
    cpack[:9, C_IDN:C_IDN + 9] = np.eye(9, dtype=np.float32)
    cpack[:9, C_ONE:C_ONE + 9] = 1.0
    cpack[0, C_X5 + 2] = 1.0
    for p in range(3):
        cpack[4 * p, C_S3 + p] = 1.0
        cpack[0:3, C_M9 + 4 * p] = 1.0
    # index->centered-coords affine: xs = px/fx + bx, ys = py/fy + by
    fx, cx = float(K[0, 0]), float(K[0, 2])
    fy, cy = float(K[1, 1]), float(K[1, 2])
    coef = (1.0 / fx, -cx / fx - float(c0x),
            1.0 / fy, -cy / fy - float(c0y))
    return Mp, cpack, float(c0x), float(c0y), coef


def _tile128(a, ntiles):
    """[ntiles*128, F] -> [128, ntiles*F] with [p, t*F+f] = a[t*128+p, f]."""
    F = a.shape[1]
    return np.ascontiguousarray(
        a.reshape(ntiles, 128, F).transpose(1, 0, 2).reshape(128, ntiles * F)
    )


DEFAULT_K = np.array([[500.0, 0.0, 320.0], [0.0, 500.0, 240.0],
                      [0.0, 0.0, 1.0]], np.float32)


def build_nc(repeats=1, no_coll=False, no_tail=False, use_f32r=True,
             dbg_c=False, c0=None, coef=None, warm=True, f32r_tail=False):
    if c0 is None or coef is None:
        _, _, c0x_, c0y_, coef = host_constants(DEFAULT_K)
        c0 = (c0x_, c0y_)
    nc = bacc.Bacc("TRN2", target_bir_lowering=False, debug=False,
                   num_devices=NCORES)

    xn = nc.dram_tensor("xn", [128, RT * N], F32, kind="ExternalInput")
    xc = nc.dram_tensor("xc", [128, RT * N], F32, kind="ExternalInput")
    m1f = nc.dram_tensor("m1f", [128, CB * 6], F32, kind="ExternalInput")
    m2s = nc.dram_tensor("m2s", [128, RT * 6], F32, kind="ExternalInput")
    cpk = nc.dram_tensor("cpack", [9, C_TOT], F32, kind="ExternalInput")
    out_d = nc.dram_tensor("out", [6, 6] if dbg_c else [3, 3], F32, kind="ExternalOutput")

    warm_in = nc.dram_tensor("warm_in", [1, 1], U8)
    warm_out = nc.dram_tensor("warm_out", [NCORES, 1], U8, addr_space="Shared")
    tr_in = nc.dram_tensor("tr_in", [1, SH], F32)
    tr_out = nc.dram_tensor("tr_out", [NCORES, SH], F32, addr_space="Shared")
    cr_in = nc.dram_tensor("cr_in", [6, 6], F32)
    cr_out = nc.dram_tensor("cr_out", [NCORES * 6, 6], F32,
                            addr_space="Shared")
    stage = nc.dram_tensor("stage", [64], F32)
    mshuf = nc.dram_tensor("mshuf", [81], F32)

    groups = [list(range(NCORES))]

    with tile.TileContext(nc) as tc:
        with (
            tc.tile_pool(name="persist", bufs=1) as pp,
            tc.tile_pool(name="scratch", bufs=2) as sp,
            tc.tile_pool(name="ps_t", bufs=2, space="PSUM") as ps,
            tc.tile_pool(name="ps_T", bufs=2, space="PSUM") as psT,
            tc.tile_pool(name="ps_c", bufs=1, space="PSUM") as psc,
        ):
            for _rep in range(repeats):
                # ---------- P0: loads ----------
                # XN thirds across sync/scalar/gpsimd (row thresholds gate
                # the AllGather trigger); XC thirds follow on the same
                # queues; consts first on scalar (tiny, unblock casts).
                XN = pp.tile([128, RT * N], F32, tag="XN")
                XC = pp.tile([128, RT * N], F32, tag="XC")
                TN = N // 3
                m1s_s = pp.tile([128, CB * 6], F32, tag="m1f")
                nc.scalar.dma_start(m1s_s[:], m1f[:])
                m2s_s = pp.tile([128, RT * 6], F32, tag="m2s")
                nc.scalar.dma_start(m2s_s[:], m2s[:])
                cps = pp.tile([9, C_TOT], F32, tag="cpk")
                nc.scalar.dma_start(cps[:], cpk[:])
                qs = [nc.sync, nc.scalar, nc.gpsimd]
                for t in range(RT):
                    a = t * N
                    for qi, q in enumerate(qs):
                        q.dma_start(XN[:, a + qi * TN:a + (qi + 1) * TN],
                                    xn[:, a + qi * TN:a + (qi + 1) * TN])
                for t in range(RT):
                    a = t * N
                    for qi, q in enumerate([nc.sync, nc.scalar, nc.scalar]):
                        q.dma_start(XC[:, a + qi * TN:a + (qi + 1) * TN],
                                    xc[:, a + qi * TN:a + (qi + 1) * TN])
                sqwarm = sp.tile([1, 1], F32, tag="sqwarm")
                nc.scalar.activation(sqwarm[:], cps[0:1, 0:1], AF.Sqrt)

                def XNt(t):
                    return XN[:, t * N:(t + 1) * N]

                def XCt(t):
                    return XC[:, t * N:(t + 1) * N]

                # ---------- P1: row thresholds -> coll1 ----------
                r8 = pp.tile([128, RT * 8], F32, tag="r8")
                for t in range(RT):
                    nc.vector.max(out=r8[:, t * 8:t * 8 + 8], in_=XNt(t))
                trT0 = pp.tile([128, RT], F32, tag="trT0")
                nc.vector.tensor_scalar_max(
                    trT0[:],
                    r8[:].rearrange("p (t e) -> p t e", e=8)[:, :, 2], T0)
                for t in range(RT):
                    nc.gpsimd.dma_start(tr_in[0:1, t * 128:(t + 1) * 128],
                                        trT0[:, t:t + 1])

                if no_coll:
                    nc.sync.dma_start(tr_out[0:1, :], tr_in[:])
                else:
                    nc.gpsimd.collective_compute(
                        "AllGather", OP.bypass, replica_groups=groups,
                        ins=[tr_in[:]], outs=[tr_out[:]])

                # ---------- P2: col thresholds (local, exact) ----------
                c8 = pp.tile([128, RT * 8], F32, tag="c8")
                for t in range(RT):
                    nc.vector.max(out=c8[:, t * 8:t * 8 + 8], in_=XCt(t))

                # ---------- P3 (pre-coll): Z mask + candidates ----------
                WDT = F32R if use_f32r else F32
                m2r = pp.tile([128, RT * 6], WDT, tag="m2r")
                nc.vector.tensor_copy(m2r[:], m2s_s[:])
                Wr = pp.tile([128, RT * N], WDT, tag="Wr")
                for t in range(RT):
                    tcl = c8[:, t * 8 + 2:t * 8 + 3]
                    nc.vector.scalar_tensor_tensor(
                        Wr[:, t * N:(t + 1) * N], XCt(t), tcl, XCt(t),
                        OP.is_ge, OP.mult)
                # candidate indices (slots 0..2 per tile)
                ci = pp.tile([128, RT * 8], U32, tag="ci")
                for t in range(RT):
                    nc.vector.max_index(out=ci[:, t * 8:t * 8 + 8],
                                        in_max=c8[:, t * 8:t * 8 + 8],
                                        in_values=XCt(t))
                # compact candidate values z9 [128, 9]
                KG = 3
                z9 = pp.tile([128, RT * KG], F32, tag="z9")
                for t in range(RT):
                    tcl = c8[:, t * 8 + 2:t * 8 + 3]
                    nc.vector.scalar_tensor_tensor(
                        z9[:, t * KG:(t + 1) * KG], c8[:, t * 8:t * 8 + KG],
                        tcl, c8[:, t * 8:t * 8 + KG], OP.is_ge, OP.mult)
                # candidate monomial coords from indices (arithmetic):
                # r = ci; px = r & 63; py = r >> 6; xs = px*ax+bx; ys = py*ay+by
                civ = ci[:].rearrange("p (t e) -> p t e", e=8)[:, :, 0:KG]
                pxu = pp.tile([128, RT * KG], U32, tag="pxu")
                nc.vector.tensor_scalar(
                    pxu[:].rearrange("p (t e) -> p t e", e=KG), civ,
                    W - 1, None, OP.bitwise_and)
                pyu = pp.tile([128, RT * KG], U32, tag="pyu")
                nc.vector.tensor_scalar(
                    pyu[:].rearrange("p (t e) -> p t e", e=KG), civ,
                    6, None, OP.logical_shift_right)
                pxf = pp.tile([128, RT * KG], F32, tag="pxf")
                nc.vector.tensor_copy(pxf[:], pxu[:])
                pyf = pp.tile([128, RT * KG], F32, tag="pyf")
                nc.vector.tensor_copy(pyf[:], pyu[:])
                xs9 = pp.tile([128, RT * KG], F32, tag="xs9")
                nc.vector.tensor_scalar(xs9[:], pxf[:], coef[0], coef[1],
                                        OP.mult, OP.add)
                ys9 = pp.tile([128, RT * KG], F32, tag="ys9")
                nc.vector.tensor_scalar(ys9[:], pyf[:], coef[2], coef[3],
                                        OP.mult, OP.add)

                # ---------- pre-coll Z-gram: T = m2'^T Z^T ----------
                m1r = pp.tile([128, CB * 6], WDT, tag="m1r")
                nc.vector.tensor_copy(m1r[:], m1s_s[:])
                Tsb = pp.tile([6, N], F32, tag="Tsb")
                TT = pp.tile([128, CB * 6], WDT, tag="TT")
                i6 = cps[0:6, C_IDN:C_IDN + 6]
                pc0 = psc.tile([6, 6], F32, tag="pc0")
                pc1 = psc.tile([6, 6], F32, tag="pc1")
                for ch in range(6):
                    Tp = psT.tile([6, 512], F32, tag="Tp")
                    for t in range(RT):
                        c0_ = t * N + ch * 512
                        nc.tensor.matmul(
                            Tp[:], m2r[:, t * 6:(t + 1) * 6],
                            Wr[:, c0_:c0_ + 512],
                            start=(t == 0), stop=(t == RT - 1))
                    nc.scalar.activation(Tsb[:, ch * 512:(ch + 1) * 512],
                                         Tp[:], AF.Copy)
                    for jj in range(4):
                        j = ch * 4 + jj
                        pt = ps.tile([128, 6], F32, tag="tps")
                        nc.tensor.transpose(
                            pt[:], Tsb[:, j * 128:(j + 1) * 128], i6)
                        nc.scalar.activation(TT[:, j * 6:(j + 1) * 6],
                                             pt[:], AF.Copy)
                for j in range(CB):
                    pc = pc0 if j < 12 else pc1
                    nc.tensor.matmul(pc[:], m1r[:, j * 6:(j + 1) * 6],
                                     TT[:, j * 6:(j + 1) * 6],
                                     start=(j % 12 == 0), stop=(j % 12 == 11))
                CpZ = pp.tile([6, 6], F32, tag="CpZ")
                nc.vector.tensor_copy(CpZ[:], pc0[:])
                nc.vector.tensor_tensor(CpZ[:], CpZ[:], pc1[:], OP.add)

                # ---------- post-coll: sparse row-test correction ----------
                # g8[p, t*3+k] = trT0full[ci[p, t*8+k]] -- 9 gathers
                g8 = pp.tile([128, RT * KG], F32, tag="g8")
                trflat = tr_out[:].rearrange("k i -> (k i)").unsqueeze(1)
                for t in range(RT):
                    for k in range(KG):
                        sl = t * KG + k
                        nc.gpsimd.indirect_dma_start(
                            out=g8[:, sl:sl + 1], out_offset=None,
                            in_=trflat,
                            in_offset=bass.IndirectOffsetOnAxis(
                                ap=ci[:, t * 8 + k:t * 8 + k + 1], axis=0))
                # fused correction: prod strips [e1x, e1y, e1, e2y, e2, d]
                # over all 9 slots; per-tile reduce feeds the gram matmuls.
                NS = RT * KG
                ccp = ps.tile([6, 6], F32, tag="tps")
                TTc = sp.tile([128, RT * 6], WDT, tag="TTc")
                prod = sp.tile([128, 6 * NS], F32, tag="prod")
                pr = prod[:].rearrange("p (c t s) -> p c t s", c=6, s=KG)

                def strip(c, t=None):
                    if t is None:
                        return prod[:, c * NS:(c + 1) * NS]
                    return pr[:, c, t, :]
                dall = strip(5)
                nc.vector.tensor_tensor(dall, z9[:], g8[:], OP.is_lt)
                nc.vector.tensor_tensor(dall, dall, z9[:], OP.mult)
                nc.vector.tensor_tensor(strip(2), dall, xs9[:], OP.mult)
                nc.vector.tensor_tensor(strip(4), dall, ys9[:], OP.mult)
                nc.vector.tensor_tensor(strip(0), strip(2), xs9[:], OP.mult)
                nc.vector.tensor_tensor(strip(1), strip(2), ys9[:], OP.mult)
                nc.vector.tensor_tensor(strip(3), strip(4), ys9[:], OP.mult)
                for t in range(RT):
                    with nc.allow_low_precision(reason="f32r is f32-width"):
                        nc.vector.tensor_reduce(
                            TTc[:, t * 6:(t + 1) * 6].unsqueeze(2),
                            pr[:, :, t, :], AX.X, OP.add)
                    nc.tensor.matmul(ccp[:], TTc[:, t * 6:(t + 1) * 6],
                                     m2r[:, t * 6:(t + 1) * 6],
                                     start=(t == 0), stop=(t == RT - 1))
                Cp = sp.tile([6, 6], F32, tag="Cp")
                nc.vector.tensor_tensor(Cp[:], CpZ[:], ccp[:], OP.subtract)
                nc.gpsimd.dma_start(cr_in[:], Cp[:])

                # ---------- coll2: AllGather 6x6 grams, sum locally ----------
                if no_coll:
                    zz = sp.tile([(NCORES - 1) * 6, 6], F32, tag="zz")
                    nc.vector.memset(zz[:], 0.0)
                    nc.sync.dma_start(cr_out[0:6, :], cr_in[:])
                    nc.sync.dma_start(cr_out[6:NCORES * 6, :], zz[:])
                else:
                    nc.gpsimd.collective_compute(
                        "AllGather", OP.bypass, replica_groups=groups,
                        ins=[cr_in[:]], outs=[cr_out[:]])

                if no_tail:
                    nn = 6 if dbg_c else 3
                    dummy = sp.tile([nn, nn], F32, tag="dummy")
                    nc.sync.dma_start(dummy[:], cr_out[0:nn, 0:nn])
                    nc.sync.dma_start(out_d[:], dummy[:])
                    continue

                # ---------- tail ----------
                _tail(nc, pp, sp, ps, psc, cps, c0, cr_out, stage, mshuf,
                      out_d, f32r_tail)

    nc.compile()
    return nc


def _transpose(nc, ps, sp, in_sb, n, idn, tag, dt=F32):
    pt = ps.tile([n, n], F32, tag="tps")
    nc.tensor.transpose(pt[:], in_sb, idn[:n, :n])
    ot = sp.tile([n, n], dt, tag=f"ot_{tag}")
    nc.vector.tensor_copy(ot[:], pt[:])
    return ot


def _powchain(nc, ps, sp, m_sb, n, tag, n_squarings=5, extra=True, dt=F32):
    """M^50 (extra=True: 5 squarings + M48=M32@M16 + M50=M48@M2) or M^32.

    Intermediates use dt (f32r halves instruction count); the returned
    final power is always F32 so downstream vector-extract matmuls stay
    within fp32r ISA restrictions.
    """
    powers = {}
    cur = m_sb
    for i in range(1, n_squarings + 1):
        last = (i == n_squarings) and not extra
        pm = ps.tile([n, n], F32, tag="tps")
        nc.tensor.matmul(pm[:], cur, cur, start=True, stop=True)
        nxt = sp.tile([n, n], F32 if last else dt, tag=f"pw_{tag}_{i}")
        nc.vector.tensor_scalar_mul(nxt[:], pm[:], 2.0)
        powers[2 ** i] = nxt
        cur = nxt[:]
    if not extra:
        return powers[2 ** n_squarings]
    pm = ps.tile([n, n], F32, tag="tps")
    nc.tensor.matmul(pm[:], powers[32][:], powers[16][:], start=True, stop=True)
    m48 = sp.tile([n, n], dt, tag=f"pw_{tag}_48")
    nc.vector.tensor_scalar_mul(m48[:], pm[:], 2.0)
    pm = ps.tile([n, n], F32, tag="tps")
    nc.tensor.matmul(pm[:], m48[:], powers[2][:], start=True, stop=True)
    m50 = sp.tile([n, n], F32, tag=f"pw_{tag}_50")
    nc.vector.tensor_scalar_mul(m50[:], pm[:], 2.0)
    return m50


def _tail(nc, pp, sp, ps, psc, cps, c0, cr_out, stage, mshuf, out_d,
          f32r_tail):
    """C' -> Hartley -> L-transform -> Mmat -> chains -> projection."""
    idn = cps[0:9, C_IDN:C_IDN + 9]

    # read gathered grams, sum over cores: CpBoth = [C'^T | C']
    csum = sp.tile([6, NCORES * 6], F32, tag="csum")
    nc.sync.dma_start(
        csum[:].rearrange("r (k c) -> r k c", c=6),
        cr_out[:].rearrange("(k r) c -> r k c", r=6))
    CpBoth = sp.tile([6, 12], F32, tag="CpBoth")
    Cp = CpBoth[:, 6:12]
    nc.vector.tensor_reduce(
        Cp.unsqueeze(2),
        csum[:].rearrange("r (k c) -> r c k", c=6), AX.X, OP.add)
    ptC = ps.tile([6, 6], F32, tag="tps")
    nc.tensor.transpose(ptC[:], Cp, cps[0:6, C_IDN:C_IDN + 6])
    nc.vector.tensor_copy(CpBoth[:, 0:6], ptC[:])

    # moments [1,12] via PE row-extract: sc[0:6]=row5(C'^T), sc[6:12]=row5(C')
    sc = pp.tile([128, 112], F32, tag="tailsc")
    e5 = cps[0:6, C_IDN + 5:C_IDN + 6]
    scm = ps.tile([1, 12], F32, tag="tps")
    nc.tensor.matmul(scm[:], e5, CpBoth[:], start=True, stop=True)
    nc.vector.tensor_copy(sc[0:1, 0:12], scm[:])

    def scv(a, b):
        return sc[0:1, a:b]

    def pair(k):
        return sc[0:1, 0:12].rearrange("p (g d) -> p d g", g=2)[:, k, :]

    Sxx, Sx, Syy, Sy, Sw = pair(0), pair(2), pair(3), pair(4), pair(5)
    ws = scv(12, 14); nc.vector.tensor_scalar_add(ws, Sw, EPS)
    rws = scv(14, 16); nc.vector.reciprocal(rws, ws)
    cx = scv(16, 18); nc.vector.tensor_tensor(cx, Sx, rws, OP.mult)  # = dx
    cy = scv(18, 20); nc.vector.tensor_tensor(cy, Sy, rws, OP.mult)  # = dy
    t_a = scv(20, 22); nc.vector.tensor_tensor(t_a, cx, Sx, OP.mult)
    t_b = scv(22, 24); nc.vector.tensor_tensor(t_b, cy, Sy, OP.mult)
    cdS = scv(24, 26); nc.vector.tensor_tensor(cdS, t_a, t_b, OP.add)
    u_a = scv(26, 28); nc.vector.tensor_tensor(u_a, cx, cx, OP.mult)
    u_b = scv(28, 30); nc.vector.tensor_tensor(u_b, cy, cy, OP.mult)
    c2_ = scv(30, 32); nc.vector.tensor_tensor(c2_, u_a, u_b, OP.add)
    sq_ = scv(32, 34); nc.vector.tensor_tensor(sq_, Sxx, Syy, OP.add)
    n2c = scv(34, 36); nc.vector.tensor_scalar_mul(n2c, cdS, -2.0)
    c2w = scv(36, 38); nc.vector.tensor_tensor(c2w, c2_, Sw, OP.mult)
    m_ = scv(38, 40); nc.vector.tensor_tensor(m_, sq_, n2c, OP.add)
    m2_ = scv(40, 42); nc.vector.tensor_tensor(m2_, m_, c2w, OP.add)
    md2 = scv(42, 44); nc.vector.tensor_tensor(md2, m2_, rws, OP.mult)
    md2e = scv(44, 46); nc.vector.tensor_scalar_add(md2e, md2, EPS)
    md = scv(46, 48); nc.scalar.activation(md, md2e, AF.Sqrt)
    mde = scv(48, 50); nc.vector.tensor_scalar_add(mde, md, EPS)
    rmd = scv(50, 52); nc.vector.reciprocal(rmd, mde)
    s_ = scv(52, 54); nc.vector.tensor_scalar_mul(s_, rmd, SQRT2)
    # real centroids: cr = dx + c0 ; then -s*cr
    cxr = scv(54, 56); nc.vector.tensor_scalar_add(cxr, cx, c0[0])
    cyr = scv(56, 58); nc.vector.tensor_scalar_add(cyr, cy, c0[1])
    scx = scv(58, 60); nc.vector.tensor_tensor(scx, s_, cxr, OP.mult)
    scy = scv(60, 62); nc.vector.tensor_tensor(scy, s_, cyr, OP.mult)
    nscx = scv(62, 64); nc.vector.tensor_scalar_mul(nscx, scx, -1.0)
    nscy = scv(64, 66); nc.vector.tensor_scalar_mul(nscy, scy, -1.0)
    # L scalars: s2, dx2, dxy, dy2 (paired)
    s2p = scv(66, 68); nc.vector.tensor_tensor(s2p, s_, s_, OP.mult)
    dx2 = scv(68, 70); nc.vector.tensor_tensor(dx2, cx, cx, OP.mult)
    dxy = scv(70, 72); nc.vector.tensor_tensor(dxy, cx, cy, OP.mult)
    dy2 = scv(72, 74); nc.vector.tensor_tensor(dy2, cy, cy, OP.mult)

    # broadcast scalar strip to 6 partitions (PE ones); consumers read PSUM
    ones16 = cps[0:1, C_ONE:C_ONE + 6]
    scBt = psc.tile([6, 80], F32, tag="scBp")
    nc.tensor.matmul(scBt[:], ones16, sc[0:1, 0:80], start=True, stop=True)
    scB = scBt

    # T row-major 9-vectors: t1v at 76:85, t2v at 85:94 (stage bounce --
    # partition-offset writes are illegal on compute engines)
    nc.vector.memset(scv(76, 94), 0.0)
    tv = sc[0:1, 76:94]
    tv9 = tv.rearrange("p (v f) -> p v f", v=2)
    nc.vector.tensor_copy(tv9[:, :, 0:1], s_.unsqueeze(2))
    nc.vector.tensor_copy(tv9[:, :, 4:5], s_.unsqueeze(2))
    nc.vector.tensor_copy(
        tv9[:, :, 2:8].rearrange("p v (c d) -> p v c d", c=2)[:, :, :, 0:1],
        sc[0:1, 62:66].rearrange("p (c v) -> p v c", c=2).unsqueeze(3))
    nc.vector.memset(tv9[:, :, 8:9], 1.0)
    nc.sync.dma_start(stage[0:18], tv)
    T12 = sp.tile([3, 6], F32, tag="T12")
    nc.sync.dma_start(
        T12[:].rearrange("i (v j) -> i v j", v=2),
        stage[0:18].rearrange("(v i j) -> i v j", i=3, j=3))

    def shT(side, tag, eng, srcB):
        """Sh^T for side (0/1): I^T + dx E1^T + dy E2^T + dx2 E3^T + ..."""
        dx = srcB[:, 16 + side:17 + side]
        dy = srcB[:, 18 + side:19 + side]
        dx2_ = srcB[:, 68 + side:69 + side]
        dxy_ = srcB[:, 70 + side:71 + side]
        dy2_ = srcB[:, 72 + side:73 + side]
        def M(i):
            return cps[0:6, C_SHT + 6 * i:C_SHT + 6 * i + 6]
        acc = sp.tile([6, 6], F32, tag=f"sh_{tag}")
        eng.scalar_tensor_tensor(acc[:], M(1), dx, M(0), OP.mult, OP.add)
        for i, sval in [(2, dy), (3, dx2_), (4, dxy_), (5, dy2_)]:
            eng.scalar_tensor_tensor(acc[:], M(i), sval, acc[:],
                                     OP.mult, OP.add)
        return acc

    Sh1T = shT(0, "1", nc.vector, scB)
    Sh2T = shT(1, "2", nc.vector, scB)
    # svec side1 as a [6,1] column (per-partition): c2m*s2 + c1m*s + c0m
    sv1c = sp.tile([6, 1], F32, tag="sv1c")
    tmp1 = sp.tile([6, 1], F32, tag="svt1")
    nc.vector.scalar_tensor_tensor(
        tmp1[:], cps[0:6, C_MSK:C_MSK + 1], scB[:, 66:67],
        cps[0:6, C_MSK + 2:C_MSK + 3], OP.mult, OP.add)
    nc.vector.scalar_tensor_tensor(
        sv1c[:], cps[0:6, C_MSK + 1:C_MSK + 2], scB[:, 52:53],
        tmp1[:], OP.mult, OP.add)
    # svec side2 as a [1,6] row on partition 0: [s2 s2 s s2 s 1]
    svr2 = sc[0:1, 96:102]
    s2v2 = sc[0:1, 67:68]
    sv2 = sc[0:1, 53:54]
    nc.vector.tensor_copy(
        svr2.rearrange("p (a b) -> p a b", a=3)[:, 0:2, 0:1],
        s2v2.unsqueeze(2).to_broadcast([1, 2, 1]))   # slots 0,2 = s2 (a-major)
    nc.vector.tensor_copy(svr2[:, 1:2], s2v2)        # slot 1 = s2
    nc.vector.tensor_copy(svr2[:, 3:4], s2v2)        # slot 3 = s2
    nc.vector.tensor_copy(svr2[:, 2:3], sv2)         # slot 2 = s
    nc.vector.tensor_copy(svr2[:, 4:5], sv2)         # slot 4 = s
    nc.vector.memset(svr2[:, 5:6], 1.0)
    sv2B = sp.tile([6, 6], F32, tag="sv2B")
    sv2Bp = ps.tile([6, 6], F32, tag="tps")
    nc.tensor.matmul(sv2Bp[:], ones16, svr2, start=True, stop=True)
    nc.vector.tensor_copy(sv2B[:], sv2Bp[:])

    # C2 = D1 Sh1 C' Sh2^T D2
    vps = ps.tile([6, 6], F32, tag="tps")
    nc.tensor.matmul(vps[:], Sh1T[:], Cp, start=True, stop=True)  # Sh1 C'
    vS = sp.tile([6, 6], F32, tag="vS")
    nc.vector.tensor_copy(vS[:], vps[:])
    vT = _transpose(nc, ps, sp, vS[:], 6, idn, "vT")
    ups = ps.tile([6, 6], F32, tag="tps")
    nc.tensor.matmul(ups[:], vT[:], Sh2T[:], start=True, stop=True)  # v Sh2^T
    # C2[r, c] = svec1[r] * u[r, c] * svec2[c]
    u1 = sp.tile([6, 6], F32, tag="u1")
    nc.vector.tensor_scalar_mul(u1[:], ups[:], sv1c[:])
    C2 = sp.tile([6, 6], F32, tag="C2")
    nc.vector.tensor_tensor(C2[:], u1[:], sv2B[:], OP.mult)
    C2T = _transpose(nc, ps, sp, C2[:], 6, idn, "c2t")

    _solve(nc, pp, sp, ps, psc, cps, idn, sc, C2[:], C2T[:], stage, mshuf,
           out_d, T12, f32r_tail)


def _solve(nc, pp, sp, ps, psc, cps, idn, sc, C2, C2T, stage, mshuf, out_d,
           T12, f32r_tail):
    PDT = F32R if f32r_tail else F32
    i9h = cps[0:9, C_I9H:C_I9H + 9]
    et69 = cps[0:6, C_ET69:C_ET69 + 9]
    i3c = cps[0:3, C_I3:C_I3 + 3]
    v09 = cps[0:9, C_V09:C_V09 + 1]
    v06 = cps[0:6, C_V06:C_V06 + 1]
    sel1 = cps[0:3, C_SEL1:C_SEL1 + 6]
    sel2 = cps[0:3, C_SEL2:C_SEL2 + 6]

    # G2 = E C2 E^T : G2[3a+b, 3c+d] = C2[pair(a,b), pair(c,d)]
    z_ps = ps.tile([6, 9], F32, tag="tps")
    nc.tensor.matmul(z_ps[:], C2T, et69, start=True, stop=True)  # C2 E^T
    Zs = sp.tile([6, 9], F32, tag="Zs")
    nc.vector.tensor_copy(Zs[:], z_ps[:])
    g_ps = ps.tile([9, 9], F32, tag="tps")
    nc.tensor.matmul(g_ps[:], et69, Zs[:], start=True, stop=True)    # E @ Z
    G2 = sp.tile([9, 9], F32, tag="G2")
    nc.vector.tensor_copy(G2[:], g_ps[:])

    # Mmat[3p+q, 3r+s] = G2[3p+r, 3q+s]: bounce via DRAM, split per
    # 3-row block so each read only waits its own write's receipt
    Mmat = sp.tile([9, 9], F32, tag="Mmat")
    for p in range(3):
        eng = nc.scalar if p == 1 else nc.sync
        eng.dma_start(mshuf[27 * p:27 * p + 27], G2[3 * p:3 * p + 3, :])
        eng.dma_start(
            Mmat[3 * p:3 * p + 3, :].rearrange("q (r s) -> q r s", s=3),
            mshuf[:].rearrange("(p q1 r s) -> p q1 r s", p=3, q1=3, r=3)
            .transpose([0, 2, 1, 3])[p])

    # lam = trace(Mmat) = sum G2[{0,4,8},{0,4,8}] -- from G2, overlapping
    # the Mmat DRAM bounce
    s3 = cps[0:9, C_S3:C_S3 + 3]
    d3ps = ps.tile([3, 9], F32, tag="tps")
    nc.tensor.matmul(d3ps[:], s3, G2[:], start=True, stop=True)
    d3 = sp.tile([3, 9], F32, tag="d3")
    nc.vector.tensor_tensor(d3[:], d3ps[:], cps[0:3, C_M9:C_M9 + 9], OP.mult)
    lam3 = sp.tile([3, 1], F32, tag="lam3")
    nc.vector.tensor_reduce(lam3[:], d3[:], AX.X, OP.add)
    lam2r = ps.tile([9, 1], F32, tag="tps")
    nc.tensor.matmul(lam2r[:], cps[0:3, C_ONE:C_ONE + 9], lam3[:],
                     start=True, stop=True)
    lam4 = sp.tile([9, 1], F32, tag="lam4")
    nc.vector.tensor_scalar_mul(lam4[:], lam2r[:], 2.0)
    inv2l = sp.tile([9, 1], F32, tag="inv2l")
    nc.vector.reciprocal(inv2l[:], lam4[:])
    # fp32r matmul needs even stationary-free: run the 9x9 chain as 10x10
    ND = 10 if f32r_tail else 9
    Msp = sp.tile([ND, ND], PDT, tag="Msp")
    if ND != 9:
        nc.vector.memset(Msp[:].bitcast(F32), 0.0)
    nc.vector.scalar_tensor_tensor(Msp[0:9, 0:9], Mmat[:], inv2l[:], i9h,
                                   OP.mult, OP.subtract)
    M50 = _powchain(nc, ps, sp, Msp[:], ND, "m9", 5, extra=True, dt=PDT)

    v09p = sp.tile([ND, 1], F32, tag="v09p")
    if ND != 9:
        nc.vector.memset(v09p[:], 0.0)
    nc.vector.tensor_copy(v09p[0:9, :], v09)
    w9ps = ps.tile([1, ND], F32, tag="tps")
    nc.tensor.matmul(w9ps[:], v09p[:], M50[:], start=True, stop=True)
    w9 = sp.tile([1, 9], F32, tag="w9")
    nc.vector.tensor_copy(w9[:], w9ps[0:1, 0:9])
    w9sq = sp.tile([1, 9], F32, tag="w9sq")
    nc.vector.tensor_tensor(w9sq[:], w9[:], w9[:], OP.mult)
    nn9 = sp.tile([1, 1], F32, tag="nn9")
    nc.vector.tensor_reduce(nn9[:], w9sq[:], AX.X, OP.add)
    sr9 = sp.tile([1, 1], F32, tag="sr9")
    nc.scalar.activation(sr9[:], nn9[:], AF.Sqrt)
    rs9 = sp.tile([1, 1], F32, tag="rs9")
    nc.vector.reciprocal(rs9[:], sr9[:])
    rs9c = psc.tile([3, 1], F32, tag="rs9c")
    nc.tensor.matmul(rs9c[:], cps[0:1, C_ONE:C_ONE + 3], rs9[:],
                     start=True, stop=True)

    # E = T2^T E_raw T1 (and E^T);  T1m/T2m preloaded in T12
    # Eraw [3,3] from w9 [1,9] via rank-1 sums: sum_b e_b (x) w9[3b:3b+3]
    # (raw; 1/||w9|| folded at the end)
    T1m = T12[:, 0:3]
    T2m = T12[:, 3:6]
    erps = ps.tile([3, 3], F32, tag="tps")
    for b in range(3):
        nc.tensor.matmul(erps[:], cps[0:1, C_X5 + 2 - b:C_X5 + 5 - b],
                         w9[0:1, 3 * b:3 * b + 3],
                         start=(b == 0), stop=(b == 2))
    Eraw = sp.tile([3, 3], F32, tag="Eraw")
    nc.vector.tensor_copy(Eraw[:], erps[:])

    a1ps = ps.tile([3, 3], F32, tag="tps")
    nc.tensor.matmul(a1ps[:], T2m, Eraw[:], start=True, stop=True)
    A1 = sp.tile([3, 3], F32, tag="A1")
    nc.vector.tensor_copy(A1[:], a1ps[:])
    A1T = _transpose(nc, ps, sp, A1[:], 3, idn, "a1t")
    etps = ps.tile([3, 3], F32, tag="tps")
    nc.tensor.matmul(etps[:], T1m, A1T[:], start=True, stop=True)
    ETs = sp.tile([3, 3], F32, tag="ETs")
    nc.vector.tensor_copy(ETs[:], etps[:])
    Es = _transpose(nc, ps, sp, ETs[:], 3, idn, "es")

    # B = E^T E ; blockdiag 6x6 chain (32 iters) for v1 (max) and v3 (min)
    bps = ps.tile([3, 3], F32, tag="tps")
    nc.tensor.matmul(bps[:], Es[:], Es[:], start=True, stop=True)
    Bm = sp.tile([3, 3], F32, tag="Bm")
    nc.vector.tensor_copy(Bm[:], bps[:])
    dg3 = sp.tile([3, 3], F32, tag="dg3")
    nc.vector.tensor_tensor(dg3[:], Bm[:], i3c, OP.mult)
    lb = sp.tile([3, 1], F32, tag="lb")
    nc.vector.tensor_reduce(lb[:], dg3[:], AX.X, OP.add)
    lbr = ps.tile([3, 1], F32, tag="tps")
    nc.tensor.matmul(lbr[:], cps[0:3, C_ONE:C_ONE + 3], lb[:],
                     start=True, stop=True)
    invlb = sp.tile([3, 1], F32, tag="invlb")
    nc.vector.reciprocal(invlb[:], lbr[:])
    Bs3 = sp.tile([3, 3], F32, tag="Bs3")
    nc.vector.tensor_scalar_mul(Bs3[:], Bm[:], invlb[:])
    IB = sp.tile([3, 3], F32, tag="IB")
    nc.vector.tensor_tensor(IB[:], i3c, Bs3[:], OP.subtract)
    bdps = ps.tile([6, 6], F32, tag="tps")
    nc.tensor.matmul(bdps[:, 0:3], sel1, Bs3[:], start=True, stop=True)
    nc.tensor.matmul(bdps[:, 3:6], sel2, IB[:], start=True, stop=True)
    BD = sp.tile([6, 6], PDT, tag="BD")
    nc.vector.tensor_copy(BD[:], bdps[:])
    BD32 = _powchain(nc, ps, sp, BD[:], 6, "m6", 5, extra=False, dt=PDT)

    w6ps = ps.tile([1, 6], F32, tag="tps")
    nc.tensor.matmul(w6ps[:], v06, BD32[:], start=True, stop=True)
    w6 = sp.tile([1, 6], F32, tag="w6")
    nc.vector.tensor_copy(w6[:], w6ps[:])
    w6sq = sp.tile([1, 6], F32, tag="w6sq")
    nc.vector.tensor_tensor(w6sq[:], w6[:], w6[:], OP.mult)
    nn6 = sp.tile([1, 2], F32, tag="nn6")
    nc.vector.tensor_reduce(nn6[:].unsqueeze(2),
                            w6sq[:].rearrange("p (g d) -> p g d", g=2), AX.X,
                            OP.add)
    sr6 = sp.tile([1, 2], F32, tag="sr6")
    nc.scalar.activation(sr6[:], nn6[:], AF.Sqrt)
    rs6 = sp.tile([1, 2], F32, tag="rs6")
    nc.vector.reciprocal(rs6[:], sr6[:])
    vv = sp.tile([1, 6], F32, tag="vv")
    nc.vector.tensor_tensor(
        vv[:].rearrange("p (g d) -> p g d", g=2),
        w6[:].rearrange("p (g d) -> p g d", g=2),
        rs6[:].unsqueeze(2).to_broadcast([1, 2, 3]), OP.mult)

    # v2 = cross(v3, v1), normalized with EPS
    aa = sp.tile([1, 6], F32, tag="aa")
    nc.vector.tensor_copy(
        aa[:].rearrange("p (r d) -> p r d", r=2),
        vv[:, 3:6].unsqueeze(1).to_broadcast([1, 2, 3]))
    bb = sp.tile([1, 6], F32, tag="bb")
    nc.vector.tensor_copy(
        bb[:].rearrange("p (r d) -> p r d", r=2),
        vv[:, 0:3].unsqueeze(1).to_broadcast([1, 2, 3]))
    cr1 = sp.tile([1, 3], F32, tag="cr1")
    nc.vector.tensor_tensor(cr1[:], aa[:, 1:4], bb[:, 2:5], OP.mult)
    cr2 = sp.tile([1, 3], F32, tag="cr2")
    nc.vector.tensor_tensor(cr2[:], aa[:, 2:5], bb[:, 1:4], OP.mult)
    v2r = sp.tile([1, 3], F32, tag="v2r")
    nc.vector.tensor_tensor(v2r[:], cr1[:], cr2[:], OP.subtract)
    v2sq = sp.tile([1, 3], F32, tag="v2sq")
    nc.vector.tensor_tensor(v2sq[:], v2r[:], v2r[:], OP.mult)
    nn2 = sp.tile([1, 1], F32, tag="nn2")
    nc.vector.tensor_reduce(nn2[:], v2sq[:], AX.X, OP.add)
    sr2 = sp.tile([1, 1], F32, tag="sr2")
    nc.scalar.activation(sr2[:], nn2[:], AF.Sqrt)
    sr2e = sp.tile([1, 1], F32, tag="sr2e")
    nc.vector.tensor_scalar_add(sr2e[:], sr2[:], EPS)
    rs2 = sp.tile([1, 1], F32, tag="rs2")
    nc.vector.reciprocal(rs2[:], sr2e[:])
    v2 = sp.tile([1, 3], F32, tag="v2")
    nc.vector.tensor_tensor(v2[:], v2r[:], rs2[:].to_broadcast([1, 3]), OP.mult)

    vvv = sp.tile([1, 6], F32, tag="vvv")
    nc.vector.tensor_copy(vvv[:, 0:3], vv[:, 0:3])
    nc.vector.tensor_copy(vvv[:, 3:6], v2[:])
    # Vr [2,3] rows from vvv halves; Vc [3,2] = Vr^T -- both via rank-1 MMs
    vrps = ps.tile([2, 3], F32, tag="tps")
    for r in range(2):
        nc.tensor.matmul(vrps[:], cps[0:1, C_X5 + 2 - r:C_X5 + 4 - r],
                         vvv[0:1, 3 * r:3 * r + 3],
                         start=(r == 0), stop=(r == 1))
    Vr = sp.tile([2, 3], F32, tag="Vr")
    nc.vector.tensor_copy(Vr[:], vrps[:])
    vcps = ps.tile([3, 2], F32, tag="tps")
    for r in range(2):
        nc.tensor.matmul(vcps[:], vvv[0:1, 3 * r:3 * r + 3],
                         cps[0:1, C_X5 + 2 - r:C_X5 + 4 - r],
                         start=(r == 0), stop=(r == 1))
    Vc = sp.tile([3, 2], F32, tag="Vc")
    nc.vector.tensor_copy(Vc[:], vcps[:])
    evps = ps.tile([2, 3], F32, tag="tps")
    nc.tensor.matmul(evps[:], Vc[:], ETs[:], start=True, stop=True)
    Evr = sp.tile([2, 3], F32, tag="Evr")
    nc.vector.tensor_copy(Evr[:], evps[:])
    evsq = sp.tile([2, 3], F32, tag="evsq")
    nc.vector.tensor_tensor(evsq[:], Evr[:], Evr[:], OP.mult)
    ss2 = sp.tile([2, 1], F32, tag="ss2")
    nc.vector.tensor_reduce(ss2[:], evsq[:], AX.X, OP.add)
    sv = sp.tile([2, 1], F32, tag="sv")
    nc.scalar.activation(sv[:], ss2[:], AF.Sqrt)
    ssum = ps.tile([2, 1], F32, tag="tps")
    nc.tensor.matmul(ssum[:], cps[0:2, C_ONE:C_ONE + 2], sv[:],
                     start=True, stop=True)
    savg = sp.tile([2, 1], F32, tag="savg")
    nc.vector.tensor_scalar_mul(savg[:], ssum[:], 0.5)
    sve = sp.tile([2, 1], F32, tag="sve")
    nc.vector.tensor_scalar_add(sve[:], sv[:], EPS)
    rsv = sp.tile([2, 1], F32, tag="rsv")
    nc.vector.reciprocal(rsv[:], sve[:])
    f2 = sp.tile([2, 1], F32, tag="f2")
    nc.vector.tensor_tensor(f2[:], rsv[:], savg[:], OP.mult)
    U2 = sp.tile([2, 3], F32, tag="U2")
    nc.vector.tensor_scalar_mul(U2[:], Evr[:], f2[:])
    ops_ = ps.tile([3, 3], F32, tag="tps")
    nc.tensor.matmul(ops_[:], U2[:], Vr[:], start=True, stop=True)
    outs = sp.tile([3, 3], F32, tag="outs")
    nc.vector.tensor_scalar_mul(outs[:], ops_[:], rs9c[:])
    nc.sync.dma_start(out_d[:], outs[:])


def make_in_maps(P, K):
    P = np.asarray(P, np.float32)
    K = np.asarray(K, np.float32)
    Pc = np.ascontiguousarray(P[:N, :N])
    PcT = np.ascontiguousarray(Pc.T)
    Mp, cpack, c0x, c0y, coef = host_constants(K)
    m1full = _tile128(Mp, CB)
    in_maps = []
    for k in range(NCORES):
        in_maps.append({
            "xn": _tile128(Pc[k * SH:(k + 1) * SH], RT),
            "xc": _tile128(PcT[k * SH:(k + 1) * SH], RT),
            "m1f": m1full,
            "m2s": _tile128(Mp[k * SH:(k + 1) * SH], RT),
            "cpack": cpack,
        })
    return in_maps


_NC_CACHE = {}


def kernel(P, K):
    from concourse.bass_utils import run_bass_kernel_spmd
    if "nc" not in _NC_CACHE:
        _, _, c0x, c0y, coef = host_constants(np.asarray(K, np.float32))
        _NC_CACHE["nc"] = build_nc(c0=(c0x, c0y), coef=coef)
    nc = _NC_CACHE["nc"]
    in_maps = make_in_maps(P, K)
    res = run_bass_kernel_spmd(nc, in_maps, core_ids=list(range(NCORES)))
    return np.asarray(res.results[0]["out"], np.float32)


# revision 39
# speedup vs baseline: 1.0444x; 1.0444x over previous
"""Trainium2 Bass kernel for nn_EssentialMatrixEstimator (v3).

Distribution (8 cores):
  - XN: natural row-shard  (384 rows x 3072 cols) -> exact row top-3 thresholds.
  - XC: transposed col-shard (384 cols x 3072 rows as [col, row]) -> exact col
    top-3 thresholds + dense masking + col-sharded gram.
  - warmup collective (1B AllGather) issued first so the NRT entry barrier +
    cc-stream init overlap the input load phase.
  - coll1: AllGather of per-core row thresholds (384 f32 -> 3072).
  - coll2: AllGather of the per-core corrected 6x6 gram (vs AllReduce: lower
    floor); summed locally.

Math: the (N*M,9) epipolar Gram collapses to the 6x6 monomial Gram C'.
Monomials are pre-centered about the host constant c0 (grid centroid), so C'
is well-conditioned; the Hartley normalization is recovered from C' moments
(row/col 5) and applied as a 6x6 L-transform C2 = L1 C' L2^T instead of a
second gram pass.  Mmat (9x9) is an index expansion of C2; min-eigvector via
50-step shifted power iteration (rescaled repeated squaring), projection via
a 32-step 6x6 blockdiag chain.

v3 vs v2: candidate monomials computed arithmetically from gathered indices
(no m1tab indirect DMAs -> gpsimd free for the 9 post-AG threshold gathers);
correction products fused into one reduce; tail avoids the stage bounce for
T12/moments (PE row-extract + direct build from the broadcast strip); most
tail matmuls run f32r single-pass; per-queue load split (sync=XN, scalar=
consts+XN+XC, gpsimd=XC) so the threshold AllGather triggers early.
"""

import os

os.environ.setdefault("JAX_PLATFORMS", "axon")

import numpy as np

import concourse.bass as bass
import concourse.bass_isa as bass_isa
import concourse.mybir as mybir
import concourse.bacc as bacc
import concourse.tile as tile

NCORES = 8
N = 3072
SH = N // NCORES          # 384 rows/cols per core
RT = SH // 128            # 3 tiles per core shard
CB = N // 128             # 24 tiles across the full dim
F32 = mybir.dt.float32
F32R = mybir.dt.float32r
U32 = mybir.dt.uint32
U8 = mybir.dt.uint8
AF = mybir.ActivationFunctionType
OP = mybir.AluOpType
AX = mybir.AxisListType

EPS = 1e-8
SQRT2 = 1.4142135623730951
INV_SQRT3 = 1.0 / 1.7320508075688772
T0 = float(np.nextafter(np.float32(0.01), np.float32(1)))  # x > 0.01 == x >= T0
H, W = 64, 64

# cpack const layout (tensor [9, C_TOT]): column ranges
C_I9H = 0      # I9 * 0.5            [9, 9]
C_ET69 = 9     # E^T selector        [6, 9]
C_I3 = 18      # I3                  [3, 3]
C_V09 = 21     # full(1/3)           [9, 1]
C_V06 = 22     # full(1/sqrt3)       [6, 1]
C_SEL1 = 23    # [I3 | 0]            [3, 6]
C_SEL2 = 29    # [0 | I3]            [3, 6]
C_SHT = 35     # Sh component mats^T: I6, E1^T..E5^T   [6, 6*6]
C_MSK = 71     # svec masks [c2m c1m c0m]  [6, 3]
C_IDN = 74     # identity 9x9        [9, 9]
C_ONE = 83     # all-ones            [9, 9]
C_X5 = 92      # [0,0,1,0,0] row 0   [1, 5]  (rank-1 row-extract bases)
C_S3 = 97      # sel {0,4,8} cols    [9, 3]  (trace-of-Mmat selector)
C_M9 = 100     # mask 1@{0,4,8}      [3, 9]
C_TOT = 109

PAIRS = [(0, 0), (0, 1), (0, 2), (1, 1), (1, 2), (2, 2)]


def _pidx():
    d = {}
    for i, (a, b) in enumerate(PAIRS):
        d[(a, b)] = i
        d[(b, a)] = i
    return d


def grid_pts(K):
    idx = np.arange(H * W, dtype=np.float32)
    pix = np.stack([idx % np.float32(W), np.floor(idx / np.float32(W))], -1)
    K_inv = np.linalg.inv(np.asarray(K, np.float32)).astype(np.float32)
    p1h = np.concatenate([pix[:N], np.ones((N, 1), np.float32)], -1)
    pts = (p1h @ K_inv.T)[:, :2].astype(np.float32)
    return pts


def host_constants(K):
    """Pre-centered monomials + packed tail constants (f32)."""
    K = np.asarray(K, np.float32)
    pts = grid_pts(K)
    x, y = pts[:, 0], pts[:, 1]
    c0x = np.float32(x.mean())
    c0y = np.float32(y.mean())
    xs = (x - c0x).astype(np.float32)
    ys = (y - c0y).astype(np.float32)
    Mp = np.stack([xs * xs, xs * ys, xs, ys * ys, ys, np.ones_like(xs)],
                  -1).astype(np.float32)

    cpack = np.zeros((9, C_TOT), np.float32)
    cpack[:9, C_I9H:C_I9H + 9] = 0.5 * np.eye(9, dtype=np.float32)
    pid = _pidx()
    for a in range(3):
        for b in range(3):
            cpack[pid[(a, b)], C_ET69 + 3 * a + b] = 1.0
    cpack[:3, C_I3:C_I3 + 3] = np.eye(3, dtype=np.float32)
    cpack[:9, C_V09] = 1.0 / 3.0
    cpack[:6, C_V06] = INV_SQRT3
    cpack[:3, C_SEL1:C_SEL1 + 3] = np.eye(3, dtype=np.float32)
    cpack[:3, C_SEL2 + 3:C_SEL2 + 6] = np.eye(3, dtype=np.float32)

    # Sh(dx,dy) = I + dx*E1 + dy*E2 + dx^2*E3 + dx*dy*E4 + dy^2*E5
    E1 = np.zeros((6, 6), np.float32)  # dx terms
    E1[0, 2] = -2.0
    E1[1, 4] = -1.0
    E1[2, 5] = -1.0
    E2 = np.zeros((6, 6), np.float32)  # dy terms
    E2[1, 2] = -1.0
    E2[3, 4] = -2.0
    E2[4, 5] = -1.0
    E3 = np.zeros((6, 6), np.float32)  # dx^2
    E3[0, 5] = 1.0
    E4 = np.zeros((6, 6), np.float32)  # dx*dy
    E4[1, 5] = 1.0
    E5 = np.zeros((6, 6), np.float32)  # dy^2
    E5[3, 5] = 1.0
    mats = [np.eye(6, dtype=np.float32), E1, E2, E3, E4, E5]
    for i, Em in enumerate(mats):
        cpack[:6, C_SHT + 6 * i:C_SHT + 6 * i + 6] = Em.T
    # svec masks: svec = [s2,s2,s,s2,s,1] = c2m*s2 + c1m*s + c0m
    cpack[:6, C_MSK + 0] = [1, 1, 0, 1, 0, 0]
    cpack[:6, C_MSK + 1] = [0, 0, 1, 0, 1, 0]
    cpack[:6, C_MSK + 2] = [0, 0, 0, 0, 0, 1]
    cpack[:9, C_IDN:C_IDN + 9] = np.eye(9, dtype=np.float32)
    cpack[:9, C_ONE:C_ONE + 9] = 1.0
    cpack[0, C_X5 + 2] = 1.0
    for p in range(3):
        cpack[4 * p, C_S3 + p] = 1.0
        cpack[0:3, C_M9 + 4 * p] = 1.0
    # index->centered-coords affine: xs = px/fx + bx, ys = py/fy + by
    fx, cx = float(K[0, 0]), float(K[0, 2])
    fy, cy = float(K[1, 1]), float(K[1, 2])
    coef = (1.0 / fx, -cx / fx - float(c0x),
            1.0 / fy, -cy / fy - float(c0y))
    return Mp, cpack, float(c0x), float(c0y), coef


def _tile128(a, ntiles):
    """[ntiles*128, F] -> [128, ntiles*F] with [p, t*F+f] = a[t*128+p, f]."""
    F = a.shape[1]
    return np.ascontiguousarray(
        a.reshape(ntiles, 128, F).transpose(1, 0, 2).reshape(128, ntiles * F)
    )


DEFAULT_K = np.array([[500.0, 0.0, 320.0], [0.0, 500.0, 240.0],
                      [0.0, 0.0, 1.0]], np.float32)


def build_nc(repeats=1, no_coll=False, no_tail=False, use_f32r=True,
             dbg_c=False, c0=None, coef=None, warm=True, f32r_tail=False):
    if c0 is None or coef is None:
        _, _, c0x_, c0y_, coef = host_constants(DEFAULT_K)
        c0 = (c0x_, c0y_)
    nc = bacc.Bacc("TRN2", target_bir_lowering=False, debug=False,
                   num_devices=NCORES)

    xn = nc.dram_tensor("xn", [128, RT * N], F32, kind="ExternalInput")
    xc = nc.dram_tensor("xc", [128, RT * N], F32, kind="ExternalInput")
    m1f = nc.dram_tensor("m1f", [128, CB * 6], F32, kind="ExternalInput")
    m2s = nc.dram_tensor("m2s", [128, RT * 6], F32, kind="ExternalInput")
    cpk = nc.dram_tensor("cpack", [9, C_TOT], F32, kind="ExternalInput")
    out_d = nc.dram_tensor("out", [6, 6] if dbg_c else [3, 3], F32, kind="ExternalOutput")

    warm_in = nc.dram_tensor("warm_in", [1, 1], U8)
    warm_out = nc.dram_tensor("warm_out", [NCORES, 1], U8, addr_space="Shared")
    tr_in = nc.dram_tensor("tr_in", [1, SH], F32)
    tr_out = nc.dram_tensor("tr_out", [NCORES, SH], F32, addr_space="Shared")
    cr_in = nc.dram_tensor("cr_in", [6, 6], F32)
    cr_out = nc.dram_tensor("cr_out", [NCORES * 6, 6], F32,
                            addr_space="Shared")
    stage = nc.dram_tensor("stage", [64], F32)
    mshuf = nc.dram_tensor("mshuf", [81], F32)

    groups = [list(range(NCORES))]

    with tile.TileContext(nc) as tc:
        with (
            tc.tile_pool(name="persist", bufs=1) as pp,
            tc.tile_pool(name="scratch", bufs=2) as sp,
            tc.tile_pool(name="ps_t", bufs=2, space="PSUM") as ps,
            tc.tile_pool(name="ps_T", bufs=2, space="PSUM") as psT,
            tc.tile_pool(name="ps_c", bufs=1, space="PSUM") as psc,
        ):
            for _rep in range(repeats):
                # ---------- P0: loads ----------
                # XN thirds across sync/scalar/gpsimd (row thresholds gate
                # the AllGather trigger); XC thirds follow on the same
                # queues; consts first on scalar (tiny, unblock casts).
                XN = pp.tile([128, RT * N], F32, tag="XN")
                XC = pp.tile([128, RT * N], F32, tag="XC")
                TN = N // 3
                m1s_s = pp.tile([128, CB * 6], F32, tag="m1f")
                nc.scalar.dma_start(m1s_s[:], m1f[:])
                m2s_s = pp.tile([128, RT * 6], F32, tag="m2s")
                nc.scalar.dma_start(m2s_s[:], m2s[:])
                cps = pp.tile([9, C_TOT], F32, tag="cpk")
                nc.scalar.dma_start(cps[:], cpk[:])
                qs = [nc.sync, nc.scalar, nc.gpsimd]
                for t in range(RT):
                    a = t * N
                    for qi, q in enumerate(qs):
                        q.dma_start(XN[:, a + qi * TN:a + (qi + 1) * TN],
                                    xn[:, a + qi * TN:a + (qi + 1) * TN])
                for t in range(RT):
                    a = t * N
                    for qi, q in enumerate([nc.sync, nc.scalar, nc.scalar]):
                        q.dma_start(XC[:, a + qi * TN:a + (qi + 1) * TN],
                                    xc[:, a + qi * TN:a + (qi + 1) * TN])
                sqwarm = sp.tile([1, 1], F32, tag="sqwarm")
                nc.scalar.activation(sqwarm[:], cps[0:1, 0:1], AF.Sqrt)

                def XNt(t):
                    return XN[:, t * N:(t + 1) * N]

                def XCt(t):
                    return XC[:, t * N:(t + 1) * N]

                # ---------- P1: row thresholds -> coll1 ----------
                r8 = pp.tile([128, RT * 8], F32, tag="r8")
                for t in range(RT):
                    nc.vector.max(out=r8[:, t * 8:t * 8 + 8], in_=XNt(t))
                trT0 = pp.tile([128, RT], F32, tag="trT0")
                nc.vector.tensor_scalar_max(
                    trT0[:],
                    r8[:].rearrange("p (t e) -> p t e", e=8)[:, :, 2], T0)
                for t in range(RT):
                    nc.gpsimd.dma_start(tr_in[0:1, t * 128:(t + 1) * 128],
                                        trT0[:, t:t + 1])

                if no_coll:
                    nc.sync.dma_start(tr_out[0:1, :], tr_in[:])
                else:
                    nc.gpsimd.collective_compute(
                        "AllGather", OP.bypass, replica_groups=groups,
                        ins=[tr_in[:]], outs=[tr_out[:]])

                # ---------- P2: col thresholds (local, exact) ----------
                c8 = pp.tile([128, RT * 8], F32, tag="c8")
                for t in range(RT):
                    nc.vector.max(out=c8[:, t * 8:t * 8 + 8], in_=XCt(t))

                # ---------- P3 (pre-coll): Z mask + candidates ----------
                WDT = F32R if use_f32r else F32
                m2r = pp.tile([128, RT * 6], WDT, tag="m2r")
                nc.vector.tensor_copy(m2r[:], m2s_s[:])
                Wr = pp.tile([128, RT * N], WDT, tag="Wr")
                for t in range(RT):
                    tcl = c8[:, t * 8 + 2:t * 8 + 3]
                    nc.vector.scalar_tensor_tensor(
                        Wr[:, t * N:(t + 1) * N], XCt(t), tcl, XCt(t),
                        OP.is_ge, OP.mult)
                # candidate indices (slots 0..2 per tile)
                ci = pp.tile([128, RT * 8], U32, tag="ci")
                for t in range(RT):
                    nc.vector.max_index(out=ci[:, t * 8:t * 8 + 8],
                                        in_max=c8[:, t * 8:t * 8 + 8],
                                        in_values=XCt(t))
                # compact candidate values z9 [128, 9]
                KG = 3
                z9 = pp.tile([128, RT * KG], F32, tag="z9")
                for t in range(RT):
                    tcl = c8[:, t * 8 + 2:t * 8 + 3]
                    nc.vector.scalar_tensor_tensor(
                        z9[:, t * KG:(t + 1) * KG], c8[:, t * 8:t * 8 + KG],
                        tcl, c8[:, t * 8:t * 8 + KG], OP.is_ge, OP.mult)
                # candidate monomial coords from indices (arithmetic):
                # r = ci; px = r & 63; py = r >> 6; xs = px*ax+bx; ys = py*ay+by
                civ = ci[:].rearrange("p (t e) -> p t e", e=8)[:, :, 0:KG]
                pxu = pp.tile([128, RT * KG], U32, tag="pxu")
                nc.vector.tensor_scalar(
                    pxu[:].rearrange("p (t e) -> p t e", e=KG), civ,
                    W - 1, None, OP.bitwise_and)
                pyu = pp.tile([128, RT * KG], U32, tag="pyu")
                nc.vector.tensor_scalar(
                    pyu[:].rearrange("p (t e) -> p t e", e=KG), civ,
                    6, None, OP.logical_shift_right)
                pxf = pp.tile([128, RT * KG], F32, tag="pxf")
                nc.vector.tensor_copy(pxf[:], pxu[:])
                pyf = pp.tile([128, RT * KG], F32, tag="pyf")
                nc.vector.tensor_copy(pyf[:], pyu[:])
                xs9 = pp.tile([128, RT * KG], F32, tag="xs9")
                nc.vector.tensor_scalar(xs9[:], pxf[:], coef[0], coef[1],
                                        OP.mult, OP.add)
                ys9 = pp.tile([128, RT * KG], F32, tag="ys9")
                nc.vector.tensor_scalar(ys9[:], pyf[:], coef[2], coef[3],
                                        OP.mult, OP.add)

                # ---------- pre-coll Z-gram: T = m2'^T Z^T ----------
                m1r = pp.tile([128, CB * 6], WDT, tag="m1r")
                nc.vector.tensor_copy(m1r[:], m1s_s[:])
                Tsb = pp.tile([6, N], F32, tag="Tsb")
                TT = pp.tile([128, CB * 6], WDT, tag="TT")
                i6 = cps[0:6, C_IDN:C_IDN + 6]
                pc0 = psc.tile([6, 6], F32, tag="pc0")
                pc1 = psc.tile([6, 6], F32, tag="pc1")
                for ch in range(6):
                    Tp = psT.tile([6, 512], F32, tag="Tp")
                    for t in range(RT):
                        c0_ = t * N + ch * 512
                        nc.tensor.matmul(
                            Tp[:], m2r[:, t * 6:(t + 1) * 6],
                            Wr[:, c0_:c0_ + 512],
                            start=(t == 0), stop=(t == RT - 1))
                    nc.scalar.activation(Tsb[:, ch * 512:(ch + 1) * 512],
                                         Tp[:], AF.Copy)
                    for jj in range(4):
                        j = ch * 4 + jj
                        pt = ps.tile([128, 6], F32, tag="tps")
                        nc.tensor.transpose(
                            pt[:], Tsb[:, j * 128:(j + 1) * 128], i6)
                        nc.scalar.activation(TT[:, j * 6:(j + 1) * 6],
                                             pt[:], AF.Copy)
                for j in range(CB):
                    pc = pc0 if j < 12 else pc1
                    nc.tensor.matmul(pc[:], m1r[:, j * 6:(j + 1) * 6],
                                     TT[:, j * 6:(j + 1) * 6],
                                     start=(j % 12 == 0), stop=(j % 12 == 11))
                CpZ = pp.tile([6, 6], F32, tag="CpZ")
                nc.vector.tensor_copy(CpZ[:], pc0[:])
                nc.vector.tensor_tensor(CpZ[:], CpZ[:], pc1[:], OP.add)

                # ---------- post-coll: sparse row-test correction ----------
                # g8[p, t*3+k] = trT0full[ci[p, t*8+k]] -- 9 gathers
                g8 = pp.tile([128, RT * KG], F32, tag="g8")
                trflat = tr_out[:].rearrange("k i -> (k i)").unsqueeze(1)
                for t in range(RT):
                    for k in range(KG):
                        sl = t * KG + k
                        nc.gpsimd.indirect_dma_start(
                            out=g8[:, sl:sl + 1], out_offset=None,
                            in_=trflat,
                            in_offset=bass.IndirectOffsetOnAxis(
                                ap=ci[:, t * 8 + k:t * 8 + k + 1], axis=0))
                # fused correction: prod strips [e1x, e1y, e1, e2y, e2, d]
                # per tile (pipelines behind that tile's three gathers)
                ccp = ps.tile([6, 6], F32, tag="tps")
                TTc = sp.tile([128, RT * 6], WDT, tag="TTc")
                for t in range(RT):
                    tsl = slice(t * KG, (t + 1) * KG)
                    prod = sp.tile([128, 6 * KG], F32, tag=f"prod{t}")

                    def strip(c):
                        return prod[:, c * KG:(c + 1) * KG]
                    d_ = strip(5)
                    nc.vector.tensor_tensor(d_, z9[:, tsl], g8[:, tsl],
                                            OP.is_lt)
                    nc.vector.tensor_tensor(d_, d_, z9[:, tsl], OP.mult)
                    nc.vector.tensor_tensor(strip(2), d_, xs9[:, tsl], OP.mult)
                    nc.vector.tensor_tensor(strip(4), d_, ys9[:, tsl], OP.mult)
                    nc.vector.tensor_tensor(strip(0), strip(2), xs9[:, tsl],
                                            OP.mult)
                    nc.vector.tensor_tensor(strip(1), strip(2), ys9[:, tsl],
                                            OP.mult)
                    nc.vector.tensor_tensor(strip(3), strip(4), ys9[:, tsl],
                                            OP.mult)
                    with nc.allow_low_precision(reason="f32r is f32-width"):
                        nc.vector.tensor_reduce(
                            TTc[:, t * 6:(t + 1) * 6].unsqueeze(2),
                            prod[:].rearrange("p (c s) -> p c s", s=KG),
                            AX.X, OP.add)
                    nc.tensor.matmul(ccp[:], TTc[:, t * 6:(t + 1) * 6],
                                     m2r[:, t * 6:(t + 1) * 6],
                                     start=(t == 0), stop=(t == RT - 1))
                Cp = sp.tile([6, 6], F32, tag="Cp")
                nc.vector.tensor_tensor(Cp[:], CpZ[:], ccp[:], OP.subtract)
                nc.gpsimd.dma_start(cr_in[:], Cp[:])

                # ---------- coll2: AllGather 6x6 grams, sum locally ----------
                if no_coll:
                    zz = sp.tile([(NCORES - 1) * 6, 6], F32, tag="zz")
                    nc.vector.memset(zz[:], 0.0)
                    nc.sync.dma_start(cr_out[0:6, :], cr_in[:])
                    nc.sync.dma_start(cr_out[6:NCORES * 6, :], zz[:])
                else:
                    nc.gpsimd.collective_compute(
                        "AllGather", OP.bypass, replica_groups=groups,
                        ins=[cr_in[:]], outs=[cr_out[:]])

                if no_tail:
                    nn = 6 if dbg_c else 3
                    dummy = sp.tile([nn, nn], F32, tag="dummy")
                    nc.sync.dma_start(dummy[:], cr_out[0:nn, 0:nn])
                    nc.sync.dma_start(out_d[:], dummy[:])
                    continue

                # ---------- tail ----------
                _tail(nc, pp, sp, ps, psc, cps, c0, cr_out, stage, mshuf,
                      out_d, f32r_tail)

    nc.compile()
    return nc


def _transpose(nc, ps, sp, in_sb, n, idn, tag, dt=F32):
    pt = ps.tile([n, n], F32, tag="tps")
    nc.tensor.transpose(pt[:], in_sb, idn[:n, :n])
    ot = sp.tile([n, n], dt, tag=f"ot_{tag}")
    nc.vector.tensor_copy(ot[:], pt[:])
    return ot


def _powchain(nc, ps, sp, m_sb, n, tag, n_squarings=5, extra=True, dt=F32):
    """M^50 (extra=True: 5 squarings + M48=M32@M16 + M50=M48@M2) or M^32.

    Intermediates use dt (f32r halves instruction count); the returned
    final power is always F32 so downstream vector-extract matmuls stay
    within fp32r ISA restrictions.
    """
    powers = {}
    cur = m_sb
    for i in range(1, n_squarings + 1):
        last = (i == n_squarings) and not extra
        pm = ps.tile([n, n], F32, tag="tps")
        nc.tensor.matmul(pm[:], cur, cur, start=True, stop=True)
        nxt = sp.tile([n, n], F32 if last else dt, tag=f"pw_{tag}_{i}")
        nc.vector.tensor_scalar_mul(nxt[:], pm[:], 2.0)
        powers[2 ** i] = nxt
        cur = nxt[:]
    if not extra:
        return powers[2 ** n_squarings]
    pm = ps.tile([n, n], F32, tag="tps")
    nc.tensor.matmul(pm[:], powers[32][:], powers[16][:], start=True, stop=True)
    m48 = sp.tile([n, n], dt, tag=f"pw_{tag}_48")
    nc.vector.tensor_scalar_mul(m48[:], pm[:], 2.0)
    pm = ps.tile([n, n], F32, tag="tps")
    nc.tensor.matmul(pm[:], m48[:], powers[2][:], start=True, stop=True)
    m50 = sp.tile([n, n], F32, tag=f"pw_{tag}_50")
    nc.vector.tensor_scalar_mul(m50[:], pm[:], 2.0)
    return m50


def _tail(nc, pp, sp, ps, psc, cps, c0, cr_out, stage, mshuf, out_d,
          f32r_tail):
    """C' -> Hartley -> L-transform -> Mmat -> chains -> projection."""
    idn = cps[0:9, C_IDN:C_IDN + 9]

    # read gathered grams, sum over cores: CpBoth = [C'^T | C']
    csum = sp.tile([6, NCORES * 6], F32, tag="csum")
    nc.sync.dma_start(
        csum[:].rearrange("r (k c) -> r k c", c=6),
        cr_out[:].rearrange("(k r) c -> r k c", r=6))
    CpBoth = sp.tile([6, 12], F32, tag="CpBoth")
    Cp = CpBoth[:, 6:12]
    nc.vector.tensor_reduce(
        Cp.unsqueeze(2),
        csum[:].rearrange("r (k c) -> r c k", c=6), AX.X, OP.add)
    ptC = ps.tile([6, 6], F32, tag="tps")
    nc.tensor.transpose(ptC[:], Cp, cps[0:6, C_IDN:C_IDN + 6])
    nc.vector.tensor_copy(CpBoth[:, 0:6], ptC[:])

    # moments [1,12] via PE row-extract: sc[0:6]=row5(C'^T), sc[6:12]=row5(C')
    sc = pp.tile([128, 112], F32, tag="tailsc")
    e5 = cps[0:6, C_IDN + 5:C_IDN + 6]
    scm = ps.tile([1, 12], F32, tag="tps")
    nc.tensor.matmul(scm[:], e5, CpBoth[:], start=True, stop=True)
    nc.vector.tensor_copy(sc[0:1, 0:12], scm[:])

    def scv(a, b):
        return sc[0:1, a:b]

    def pair(k):
        return sc[0:1, 0:12].rearrange("p (g d) -> p d g", g=2)[:, k, :]

    Sxx, Sx, Syy, Sy, Sw = pair(0), pair(2), pair(3), pair(4), pair(5)
    ws = scv(12, 14); nc.vector.tensor_scalar_add(ws, Sw, EPS)
    rws = scv(14, 16); nc.vector.reciprocal(rws, ws)
    cx = scv(16, 18); nc.vector.tensor_tensor(cx, Sx, rws, OP.mult)  # = dx
    cy = scv(18, 20); nc.vector.tensor_tensor(cy, Sy, rws, OP.mult)  # = dy
    t_a = scv(20, 22); nc.vector.tensor_tensor(t_a, cx, Sx, OP.mult)
    t_b = scv(22, 24); nc.vector.tensor_tensor(t_b, cy, Sy, OP.mult)
    cdS = scv(24, 26); nc.vector.tensor_tensor(cdS, t_a, t_b, OP.add)
    u_a = scv(26, 28); nc.vector.tensor_tensor(u_a, cx, cx, OP.mult)
    u_b = scv(28, 30); nc.vector.tensor_tensor(u_b, cy, cy, OP.mult)
    c2_ = scv(30, 32); nc.vector.tensor_tensor(c2_, u_a, u_b, OP.add)
    sq_ = scv(32, 34); nc.vector.tensor_tensor(sq_, Sxx, Syy, OP.add)
    n2c = scv(34, 36); nc.vector.tensor_scalar_mul(n2c, cdS, -2.0)
    c2w = scv(36, 38); nc.vector.tensor_tensor(c2w, c2_, Sw, OP.mult)
    m_ = scv(38, 40); nc.vector.tensor_tensor(m_, sq_, n2c, OP.add)
    m2_ = scv(40, 42); nc.vector.tensor_tensor(m2_, m_, c2w, OP.add)
    md2 = scv(42, 44); nc.vector.tensor_tensor(md2, m2_, rws, OP.mult)
    md2e = scv(44, 46); nc.vector.tensor_scalar_add(md2e, md2, EPS)
    md = scv(46, 48); nc.scalar.activation(md, md2e, AF.Sqrt)
    mde = scv(48, 50); nc.vector.tensor_scalar_add(mde, md, EPS)
    rmd = scv(50, 52); nc.vector.reciprocal(rmd, mde)
    s_ = scv(52, 54); nc.vector.tensor_scalar_mul(s_, rmd, SQRT2)
    # real centroids: cr = dx + c0 ; then -s*cr
    cxr = scv(54, 56); nc.vector.tensor_scalar_add(cxr, cx, c0[0])
    cyr = scv(56, 58); nc.vector.tensor_scalar_add(cyr, cy, c0[1])
    scx = scv(58, 60); nc.vector.tensor_tensor(scx, s_, cxr, OP.mult)
    scy = scv(60, 62); nc.vector.tensor_tensor(scy, s_, cyr, OP.mult)
    nscx = scv(62, 64); nc.vector.tensor_scalar_mul(nscx, scx, -1.0)
    nscy = scv(64, 66); nc.vector.tensor_scalar_mul(nscy, scy, -1.0)
    # L scalars: s2, dx2, dxy, dy2 (paired)
    s2p = scv(66, 68); nc.vector.tensor_tensor(s2p, s_, s_, OP.mult)
    dx2 = scv(68, 70); nc.vector.tensor_tensor(dx2, cx, cx, OP.mult)
    dxy = scv(70, 72); nc.vector.tensor_tensor(dxy, cx, cy, OP.mult)
    dy2 = scv(72, 74); nc.vector.tensor_tensor(dy2, cy, cy, OP.mult)

    # broadcast scalar strip to 6 partitions (PE ones); consumers read PSUM
    ones16 = cps[0:1, C_ONE:C_ONE + 6]
    scBt = psc.tile([6, 80], F32, tag="scBp")
    nc.tensor.matmul(scBt[:], ones16, sc[0:1, 0:80], start=True, stop=True)
    scB = scBt

    # T row-major 9-vectors: t1v at 76:85, t2v at 85:94 (stage bounce --
    # partition-offset writes are illegal on compute engines)
    nc.vector.memset(scv(76, 94), 0.0)
    tv = sc[0:1, 76:94]
    tv9 = tv.rearrange("p (v f) -> p v f", v=2)
    nc.vector.tensor_copy(tv9[:, :, 0:1], s_.unsqueeze(2))
    nc.vector.tensor_copy(tv9[:, :, 4:5], s_.unsqueeze(2))
    nc.vector.tensor_copy(
        tv9[:, :, 2:8].rearrange("p v (c d) -> p v c d", c=2)[:, :, :, 0:1],
        sc[0:1, 62:66].rearrange("p (c v) -> p v c", c=2).unsqueeze(3))
    nc.vector.memset(tv9[:, :, 8:9], 1.0)
    nc.sync.dma_start(stage[0:18], tv)
    T12 = sp.tile([3, 6], F32, tag="T12")
    nc.sync.dma_start(
        T12[:].rearrange("i (v j) -> i v j", v=2),
        stage[0:18].rearrange("(v i j) -> i v j", i=3, j=3))

    def shT(side, tag, eng, srcB):
        """Sh^T for side (0/1): I^T + dx E1^T + dy E2^T + dx2 E3^T + ..."""
        dx = srcB[:, 16 + side:17 + side]
        dy = srcB[:, 18 + side:19 + side]
        dx2_ = srcB[:, 68 + side:69 + side]
        dxy_ = srcB[:, 70 + side:71 + side]
        dy2_ = srcB[:, 72 + side:73 + side]
        def M(i):
            return cps[0:6, C_SHT + 6 * i:C_SHT + 6 * i + 6]
        acc = sp.tile([6, 6], F32, tag=f"sh_{tag}")
        eng.scalar_tensor_tensor(acc[:], M(1), dx, M(0), OP.mult, OP.add)
        for i, sval in [(2, dy), (3, dx2_), (4, dxy_), (5, dy2_)]:
            eng.scalar_tensor_tensor(acc[:], M(i), sval, acc[:],
                                     OP.mult, OP.add)
        return acc

    Sh1T = shT(0, "1", nc.vector, scB)
    Sh2T = shT(1, "2", nc.vector, scB)
    # svec side1 as a [6,1] column (per-partition): c2m*s2 + c1m*s + c0m
    sv1c = sp.tile([6, 1], F32, tag="sv1c")
    tmp1 = sp.tile([6, 1], F32, tag="svt1")
    nc.vector.scalar_tensor_tensor(
        tmp1[:], cps[0:6, C_MSK:C_MSK + 1], scB[:, 66:67],
        cps[0:6, C_MSK + 2:C_MSK + 3], OP.mult, OP.add)
    nc.vector.scalar_tensor_tensor(
        sv1c[:], cps[0:6, C_MSK + 1:C_MSK + 2], scB[:, 52:53],
        tmp1[:], OP.mult, OP.add)
    # svec side2 as a [1,6] row on partition 0: [s2 s2 s s2 s 1]
    svr2 = sc[0:1, 96:102]
    s2v2 = sc[0:1, 67:68]
    sv2 = sc[0:1, 53:54]
    nc.vector.tensor_copy(
        svr2.rearrange("p (a b) -> p a b", a=3)[:, 0:2, 0:1],
        s2v2.unsqueeze(2).to_broadcast([1, 2, 1]))   # slots 0,2 = s2 (a-major)
    nc.vector.tensor_copy(svr2[:, 1:2], s2v2)        # slot 1 = s2
    nc.vector.tensor_copy(svr2[:, 3:4], s2v2)        # slot 3 = s2
    nc.vector.tensor_copy(svr2[:, 2:3], sv2)         # slot 2 = s
    nc.vector.tensor_copy(svr2[:, 4:5], sv2)         # slot 4 = s
    nc.vector.memset(svr2[:, 5:6], 1.0)
    sv2B = sp.tile([6, 6], F32, tag="sv2B")
    sv2Bp = ps.tile([6, 6], F32, tag="tps")
    nc.tensor.matmul(sv2Bp[:], ones16, svr2, start=True, stop=True)
    nc.vector.tensor_copy(sv2B[:], sv2Bp[:])

    # C2 = D1 Sh1 C' Sh2^T D2
    vps = ps.tile([6, 6], F32, tag="tps")
    nc.tensor.matmul(vps[:], Sh1T[:], Cp, start=True, stop=True)  # Sh1 C'
    vS = sp.tile([6, 6], F32, tag="vS")
    nc.vector.tensor_copy(vS[:], vps[:])
    vT = _transpose(nc, ps, sp, vS[:], 6, idn, "vT")
    ups = ps.tile([6, 6], F32, tag="tps")
    nc.tensor.matmul(ups[:], vT[:], Sh2T[:], start=True, stop=True)  # v Sh2^T
    # C2[r, c] = svec1[r] * u[r, c] * svec2[c]
    u1 = sp.tile([6, 6], F32, tag="u1")
    nc.vector.tensor_scalar_mul(u1[:], ups[:], sv1c[:])
    C2 = sp.tile([6, 6], F32, tag="C2")
    nc.vector.tensor_tensor(C2[:], u1[:], sv2B[:], OP.mult)
    C2T = _transpose(nc, ps, sp, C2[:], 6, idn, "c2t")

    _solve(nc, pp, sp, ps, psc, cps, idn, sc, C2[:], C2T[:], stage, mshuf,
           out_d, T12, f32r_tail)


def _solve(nc, pp, sp, ps, psc, cps, idn, sc, C2, C2T, stage, mshuf, out_d,
           T12, f32r_tail):
    PDT = F32R if f32r_tail else F32
    i9h = cps[0:9, C_I9H:C_I9H + 9]
    et69 = cps[0:6, C_ET69:C_ET69 + 9]
    i3c = cps[0:3, C_I3:C_I3 + 3]
    v09 = cps[0:9, C_V09:C_V09 + 1]
    v06 = cps[0:6, C_V06:C_V06 + 1]
    sel1 = cps[0:3, C_SEL1:C_SEL1 + 6]
    sel2 = cps[0:3, C_SEL2:C_SEL2 + 6]

    # G2 = E C2 E^T : G2[3a+b, 3c+d] = C2[pair(a,b), pair(c,d)]
    z_ps = ps.tile([6, 9], F32, tag="tps")
    nc.tensor.matmul(z_ps[:], C2T, et69, start=True, stop=True)  # C2 E^T
    Zs = sp.tile([6, 9], F32, tag="Zs")
    nc.vector.tensor_copy(Zs[:], z_ps[:])
    g_ps = ps.tile([9, 9], F32, tag="tps")
    nc.tensor.matmul(g_ps[:], et69, Zs[:], start=True, stop=True)    # E @ Z
    G2 = sp.tile([9, 9], F32, tag="G2")
    nc.vector.tensor_copy(G2[:], g_ps[:])

    # Mmat[3p+q, 3r+s] = G2[3p+r, 3q+s]: bounce via DRAM, split per
    # 3-row block so each read only waits its own write's receipt
    Mmat = sp.tile([9, 9], F32, tag="Mmat")
    for p in range(3):
        eng = nc.scalar if p == 1 else nc.sync
        eng.dma_start(mshuf[27 * p:27 * p + 27], G2[3 * p:3 * p + 3, :])
        eng.dma_start(
            Mmat[3 * p:3 * p + 3, :].rearrange("q (r s) -> q r s", s=3),
            mshuf[:].rearrange("(p q1 r s) -> p q1 r s", p=3, q1=3, r=3)
            .transpose([0, 2, 1, 3])[p])

    # lam = trace(Mmat) = sum G2[{0,4,8},{0,4,8}] -- from G2, overlapping
    # the Mmat DRAM bounce
    s3 = cps[0:9, C_S3:C_S3 + 3]
    d3ps = ps.tile([3, 9], F32, tag="tps")
    nc.tensor.matmul(d3ps[:], s3, G2[:], start=True, stop=True)
    d3 = sp.tile([3, 9], F32, tag="d3")
    nc.vector.tensor_tensor(d3[:], d3ps[:], cps[0:3, C_M9:C_M9 + 9], OP.mult)
    lam3 = sp.tile([3, 1], F32, tag="lam3")
    nc.vector.tensor_reduce(lam3[:], d3[:], AX.X, OP.add)
    lam2r = ps.tile([9, 1], F32, tag="tps")
    nc.tensor.matmul(lam2r[:], cps[0:3, C_ONE:C_ONE + 9], lam3[:],
                     start=True, stop=True)
    lam4 = sp.tile([9, 1], F32, tag="lam4")
    nc.vector.tensor_scalar_mul(lam4[:], lam2r[:], 2.0)
    inv2l = sp.tile([9, 1], F32, tag="inv2l")
    nc.vector.reciprocal(inv2l[:], lam4[:])
    # fp32r matmul needs even stationary-free: run the 9x9 chain as 10x10
    ND = 10 if f32r_tail else 9
    Msp = sp.tile([ND, ND], PDT, tag="Msp")
    if ND != 9:
        nc.vector.memset(Msp[:].bitcast(F32), 0.0)
    nc.vector.scalar_tensor_tensor(Msp[0:9, 0:9], Mmat[:], inv2l[:], i9h,
                                   OP.mult, OP.subtract)
    M50 = _powchain(nc, ps, sp, Msp[:], ND, "m9", 5, extra=True, dt=PDT)

    v09p = sp.tile([ND, 1], F32, tag="v09p")
    if ND != 9:
        nc.vector.memset(v09p[:], 0.0)
    nc.vector.tensor_copy(v09p[0:9, :], v09)
    w9ps = ps.tile([1, ND], F32, tag="tps")
    nc.tensor.matmul(w9ps[:], v09p[:], M50[:], start=True, stop=True)
    w9 = sp.tile([1, 9], F32, tag="w9")
    nc.vector.tensor_copy(w9[:], w9ps[0:1, 0:9])
    w9sq = sp.tile([1, 9], F32, tag="w9sq")
    nc.vector.tensor_tensor(w9sq[:], w9[:], w9[:], OP.mult)
    nn9 = sp.tile([1, 1], F32, tag="nn9")
    nc.vector.tensor_reduce(nn9[:], w9sq[:], AX.X, OP.add)
    sr9 = sp.tile([1, 1], F32, tag="sr9")
    nc.scalar.activation(sr9[:], nn9[:], AF.Sqrt)
    rs9 = sp.tile([1, 1], F32, tag="rs9")
    nc.vector.reciprocal(rs9[:], sr9[:])
    rs9c = psc.tile([3, 1], F32, tag="rs9c")
    nc.tensor.matmul(rs9c[:], cps[0:1, C_ONE:C_ONE + 3], rs9[:],
                     start=True, stop=True)

    # E = T2^T E_raw T1 (and E^T);  T1m/T2m preloaded in T12
    # Eraw [3,3] from w9 [1,9] via rank-1 sums: sum_b e_b (x) w9[3b:3b+3]
    # (raw; 1/||w9|| folded at the end)
    T1m = T12[:, 0:3]
    T2m = T12[:, 3:6]
    erps = ps.tile([3, 3], F32, tag="tps")
    for b in range(3):
        nc.tensor.matmul(erps[:], cps[0:1, C_X5 + 2 - b:C_X5 + 5 - b],
                         w9[0:1, 3 * b:3 * b + 3],
                         start=(b == 0), stop=(b == 2))
    Eraw = sp.tile([3, 3], F32, tag="Eraw")
    nc.vector.tensor_copy(Eraw[:], erps[:])

    a1ps = ps.tile([3, 3], F32, tag="tps")
    nc.tensor.matmul(a1ps[:], T2m, Eraw[:], start=True, stop=True)
    A1 = sp.tile([3, 3], F32, tag="A1")
    nc.vector.tensor_copy(A1[:], a1ps[:])
    A1T = _transpose(nc, ps, sp, A1[:], 3, idn, "a1t")
    etps = ps.tile([3, 3], F32, tag="tps")
    nc.tensor.matmul(etps[:], T1m, A1T[:], start=True, stop=True)
    ETs = sp.tile([3, 3], F32, tag="ETs")
    nc.vector.tensor_copy(ETs[:], etps[:])
    Es = _transpose(nc, ps, sp, ETs[:], 3, idn, "es")

    # B = E^T E ; blockdiag 6x6 chain (32 iters) for v1 (max) and v3 (min)
    bps = ps.tile([3, 3], F32, tag="tps")
    nc.tensor.matmul(bps[:], Es[:], Es[:], start=True, stop=True)
    Bm = sp.tile([3, 3], F32, tag="Bm")
    nc.vector.tensor_copy(Bm[:], bps[:])
    dg3 = sp.tile([3, 3], F32, tag="dg3")
    nc.vector.tensor_tensor(dg3[:], Bm[:], i3c, OP.mult)
    lb = sp.tile([3, 1], F32, tag="lb")
    nc.vector.tensor_reduce(lb[:], dg3[:], AX.X, OP.add)
    lbr = ps.tile([3, 1], F32, tag="tps")
    nc.tensor.matmul(lbr[:], cps[0:3, C_ONE:C_ONE + 3], lb[:],
                     start=True, stop=True)
    invlb = sp.tile([3, 1], F32, tag="invlb")
    nc.vector.reciprocal(invlb[:], lbr[:])
    Bs3 = sp.tile([3, 3], F32, tag="Bs3")
    nc.vector.tensor_scalar_mul(Bs3[:], Bm[:], invlb[:])
    IB = sp.tile([3, 3], F32, tag="IB")
    nc.vector.tensor_tensor(IB[:], i3c, Bs3[:], OP.subtract)
    bdps = ps.tile([6, 6], F32, tag="tps")
    nc.tensor.matmul(bdps[:, 0:3], sel1, Bs3[:], start=True, stop=True)
    nc.tensor.matmul(bdps[:, 3:6], sel2, IB[:], start=True, stop=True)
    BD = sp.tile([6, 6], PDT, tag="BD")
    nc.vector.tensor_copy(BD[:], bdps[:])
    BD32 = _powchain(nc, ps, sp, BD[:], 6, "m6", 5, extra=False, dt=PDT)

    w6ps = ps.tile([1, 6], F32, tag="tps")
    nc.tensor.matmul(w6ps[:], v06, BD32[:], start=True, stop=True)
    w6 = sp.tile([1, 6], F32, tag="w6")
    nc.vector.tensor_copy(w6[:], w6ps[:])
    w6sq = sp.tile([1, 6], F32, tag="w6sq")
    nc.vector.tensor_tensor(w6sq[:], w6[:], w6[:], OP.mult)
    nn6 = sp.tile([1, 2], F32, tag="nn6")
    nc.vector.tensor_reduce(nn6[:].unsqueeze(2),
                            w6sq[:].rearrange("p (g d) -> p g d", g=2), AX.X,
                            OP.add)
    sr6 = sp.tile([1, 2], F32, tag="sr6")
    nc.scalar.activation(sr6[:], nn6[:], AF.Sqrt)
    rs6 = sp.tile([1, 2], F32, tag="rs6")
    nc.vector.reciprocal(rs6[:], sr6[:])
    vv = sp.tile([1, 6], F32, tag="vv")
    nc.vector.tensor_tensor(
        vv[:].rearrange("p (g d) -> p g d", g=2),
        w6[:].rearrange("p (g d) -> p g d", g=2),
        rs6[:].unsqueeze(2).to_broadcast([1, 2, 3]), OP.mult)

    # v2 = cross(v3, v1), normalized with EPS
    aa = sp.tile([1, 6], F32, tag="aa")
    nc.vector.tensor_copy(
        aa[:].rearrange("p (r d) -> p r d", r=2),
        vv[:, 3:6].unsqueeze(1).to_broadcast([1, 2, 3]))
    bb = sp.tile([1, 6], F32, tag="bb")
    nc.vector.tensor_copy(
        bb[:].rearrange("p (r d) -> p r d", r=2),
        vv[:, 0:3].unsqueeze(1).to_broadcast([1, 2, 3]))
    cr1 = sp.tile([1, 3], F32, tag="cr1")
    nc.vector.tensor_tensor(cr1[:], aa[:, 1:4], bb[:, 2:5], OP.mult)
    cr2 = sp.tile([1, 3], F32, tag="cr2")
    nc.vector.tensor_tensor(cr2[:], aa[:, 2:5], bb[:, 1:4], OP.mult)
    v2r = sp.tile([1, 3], F32, tag="v2r")
    nc.vector.tensor_tensor(v2r[:], cr1[:], cr2[:], OP.subtract)
    v2sq = sp.tile([1, 3], F32, tag="v2sq")
    nc.vector.tensor_tensor(v2sq[:], v2r[:], v2r[:], OP.mult)
    nn2 = sp.tile([1, 1], F32, tag="nn2")
    nc.vector.tensor_reduce(nn2[:], v2sq[:], AX.X, OP.add)
    sr2 = sp.tile([1, 1], F32, tag="sr2")
    nc.scalar.activation(sr2[:], nn2[:], AF.Sqrt)
    sr2e = sp.tile([1, 1], F32, tag="sr2e")
    nc.vector.tensor_scalar_add(sr2e[:], sr2[:], EPS)
    rs2 = sp.tile([1, 1], F32, tag="rs2")
    nc.vector.reciprocal(rs2[:], sr2e[:])
    v2 = sp.tile([1, 3], F32, tag="v2")
    nc.vector.tensor_tensor(v2[:], v2r[:], rs2[:].to_broadcast([1, 3]), OP.mult)

    vvv = sp.tile([1, 6], F32, tag="vvv")
    nc.vector.tensor_copy(vvv[:, 0:3], vv[:, 0:3])
    nc.vector.tensor_copy(vvv[:, 3:6], v2[:])
    # Vr [2,3] rows from vvv halves; Vc [3,2] = Vr^T -- both via rank-1 MMs
    vrps = ps.tile([2, 3], F32, tag="tps")
    for r in range(2):
        nc.tensor.matmul(vrps[:], cps[0:1, C_X5 + 2 - r:C_X5 + 4 - r],
                         vvv[0:1, 3 * r:3 * r + 3],
                         start=(r == 0), stop=(r == 1))
    Vr = sp.tile([2, 3], F32, tag="Vr")
    nc.vector.tensor_copy(Vr[:], vrps[:])
    vcps = ps.tile([3, 2], F32, tag="tps")
    for r in range(2):
        nc.tensor.matmul(vcps[:], vvv[0:1, 3 * r:3 * r + 3],
                         cps[0:1, C_X5 + 2 - r:C_X5 + 4 - r],
                         start=(r == 0), stop=(r == 1))
    Vc = sp.tile([3, 2], F32, tag="Vc")
    nc.vector.tensor_copy(Vc[:], vcps[:])
    evps = ps.tile([2, 3], F32, tag="tps")
    nc.tensor.matmul(evps[:], Vc[:], ETs[:], start=True, stop=True)
    Evr = sp.tile([2, 3], F32, tag="Evr")
    nc.vector.tensor_copy(Evr[:], evps[:])
    evsq = sp.tile([2, 3], F32, tag="evsq")
    nc.vector.tensor_tensor(evsq[:], Evr[:], Evr[:], OP.mult)
    ss2 = sp.tile([2, 1], F32, tag="ss2")
    nc.vector.tensor_reduce(ss2[:], evsq[:], AX.X, OP.add)
    sv = sp.tile([2, 1], F32, tag="sv")
    nc.scalar.activation(sv[:], ss2[:], AF.Sqrt)
    ssum = ps.tile([2, 1], F32, tag="tps")
    nc.tensor.matmul(ssum[:], cps[0:2, C_ONE:C_ONE + 2], sv[:],
                     start=True, stop=True)
    savg = sp.tile([2, 1], F32, tag="savg")
    nc.vector.tensor_scalar_mul(savg[:], ssum[:], 0.5)
    sve = sp.tile([2, 1], F32, tag="sve")
    nc.vector.tensor_scalar_add(sve[:], sv[:], EPS)
    rsv = sp.tile([2, 1], F32, tag="rsv")
    nc.vector.reciprocal(rsv[:], sve[:])
    f2 = sp.tile([2, 1], F32, tag="f2")
    nc.vector.tensor_tensor(f2[:], rsv[:], savg[:], OP.mult)
    U2 = sp.tile([2, 3], F32, tag="U2")
    nc.vector.tensor_scalar_mul(U2[:], Evr[:], f2[:])
    ops_ = ps.tile([3, 3], F32, tag="tps")
    nc.tensor.matmul(ops_[:], U2[:], Vr[:], start=True, stop=True)
    outs = sp.tile([3, 3], F32, tag="outs")
    nc.vector.tensor_scalar_mul(outs[:], ops_[:], rs9c[:])
    nc.sync.dma_start(out_d[:], outs[:])


def make_in_maps(P, K):
    P = np.asarray(P, np.float32)
    K = np.asarray(K, np.float32)
    Pc = np.ascontiguousarray(P[:N, :N])
    PcT = np.ascontiguousarray(Pc.T)
    Mp, cpack, c0x, c0y, coef = host_constants(K)
    m1full = _tile128(Mp, CB)
    in_maps = []
    for k in range(NCORES):
        in_maps.append({
            "xn": _tile128(Pc[k * SH:(k + 1) * SH], RT),
            "xc": _tile128(PcT[k * SH:(k + 1) * SH], RT),
            "m1f": m1full,
            "m2s": _tile128(Mp[k * SH:(k + 1) * SH], RT),
            "cpack": cpack,
        })
    return in_maps


_NC_CACHE = {}


def kernel(P, K):
    from concourse.bass_utils import run_bass_kernel_spmd
    if "nc" not in _NC_CACHE:
        _, _, c0x, c0y, coef = host_constants(np.asarray(K, np.float32))
        _NC_CACHE["nc"] = build_nc(c0=(c0x, c0y), coef=coef)
    nc = _NC_CACHE["nc"]
    in_maps = make_in_maps(P, K)
    res = run_bass_kernel_spmd(nc, in_maps, core_ids=list(range(NCORES)))
    return np.asarray(res.results[0]["out"], np.float32)


# revision 41
# speedup vs baseline: 1.1471x; 1.0983x over previous
"""Trainium2 Bass kernel for nn_EssentialMatrixEstimator (v3).

Distribution (8 cores):
  - XN: natural row-shard  (384 rows x 3072 cols) -> exact row top-3 thresholds.
  - XC: transposed col-shard (384 cols x 3072 rows as [col, row]) -> exact col
    top-3 thresholds + dense masking + col-sharded gram.
  - warmup collective (1B AllGather) issued first so the NRT entry barrier +
    cc-stream init overlap the input load phase.
  - coll1: AllGather of per-core row thresholds (384 f32 -> 3072).
  - coll2: AllGather of the per-core corrected 6x6 gram (vs AllReduce: lower
    floor); summed locally.

Math: the (N*M,9) epipolar Gram collapses to the 6x6 monomial Gram C'.
Monomials are pre-centered about the host constant c0 (grid centroid), so C'
is well-conditioned; the Hartley normalization is recovered from C' moments
(row/col 5) and applied as a 6x6 L-transform C2 = L1 C' L2^T instead of a
second gram pass.  Mmat (9x9) is an index expansion of C2; min-eigvector via
50-step shifted power iteration (rescaled repeated squaring), projection via
a 32-step 6x6 blockdiag chain.

v3 vs v2: candidate monomials computed arithmetically from gathered indices
(no m1tab indirect DMAs -> gpsimd free for the 9 post-AG threshold gathers);
correction products fused into one reduce; tail avoids the stage bounce for
T12/moments (PE row-extract + direct build from the broadcast strip); most
tail matmuls run f32r single-pass; per-queue load split (sync=XN, scalar=
consts+XN+XC, gpsimd=XC) so the threshold AllGather triggers early.
"""

import os

os.environ.setdefault("JAX_PLATFORMS", "axon")

import numpy as np

import concourse.bass as bass
import concourse.bass_isa as bass_isa
import concourse.mybir as mybir
import concourse.bacc as bacc
import concourse.tile as tile

NCORES = 8
N = 3072
SH = N // NCORES          # 384 rows/cols per core
RT = SH // 128            # 3 tiles per core shard
CB = N // 128             # 24 tiles across the full dim
F32 = mybir.dt.float32
F32R = mybir.dt.float32r
U32 = mybir.dt.uint32
U8 = mybir.dt.uint8
AF = mybir.ActivationFunctionType
OP = mybir.AluOpType
AX = mybir.AxisListType

EPS = 1e-8
SQRT2 = 1.4142135623730951
INV_SQRT3 = 1.0 / 1.7320508075688772
T0 = float(np.nextafter(np.float32(0.01), np.float32(1)))  # x > 0.01 == x >= T0
H, W = 64, 64

# cpack const layout (tensor [9, C_TOT]): column ranges
C_I9H = 0      # I9 * 0.5            [9, 9]
C_ET69 = 9     # E^T selector        [6, 9]
C_I3 = 18      # I3                  [3, 3]
C_V09 = 21     # full(1/3)           [9, 1]
C_V06 = 22     # full(1/sqrt3)       [6, 1]
C_SEL1 = 23    # [I3 | 0]            [3, 6]
C_SEL2 = 29    # [0 | I3]            [3, 6]
C_SHT = 35     # Sh component mats^T: I6, E1^T..E5^T   [6, 6*6]
C_MSK = 71     # svec masks [c2m c1m c0m]  [6, 3]
C_IDN = 74     # identity 9x9        [9, 9]
C_ONE = 83     # all-ones            [9, 9]
C_X5 = 92      # [0,0,1,0,0] row 0   [1, 5]  (rank-1 row-extract bases)
C_S3 = 97      # sel {0,4,8} cols    [9, 3]  (trace-of-Mmat selector)
C_M9 = 100     # mask 1@{0,4,8}      [3, 9]
C_TOT = 109

PAIRS = [(0, 0), (0, 1), (0, 2), (1, 1), (1, 2), (2, 2)]


def _pidx():
    d = {}
    for i, (a, b) in enumerate(PAIRS):
        d[(a, b)] = i
        d[(b, a)] = i
    return d


def grid_pts(K):
    idx = np.arange(H * W, dtype=np.float32)
    pix = np.stack([idx % np.float32(W), np.floor(idx / np.float32(W))], -1)
    K_inv = np.linalg.inv(np.asarray(K, np.float32)).astype(np.float32)
    p1h = np.concatenate([pix[:N], np.ones((N, 1), np.float32)], -1)
    pts = (p1h @ K_inv.T)[:, :2].astype(np.float32)
    return pts


def host_constants(K):
    """Pre-centered monomials + packed tail constants (f32)."""
    K = np.asarray(K, np.float32)
    pts = grid_pts(K)
    x, y = pts[:, 0], pts[:, 1]
    c0x = np.float32(x.mean())
    c0y = np.float32(y.mean())
    xs = (x - c0x).astype(np.float32)
    ys = (y - c0y).astype(np.float32)
    Mp = np.stack([xs * xs, xs * ys, xs, ys * ys, ys, np.ones_like(xs)],
                  -1).astype(np.float32)

    cpack = np.zeros((9, C_TOT), np.float32)
    cpack[:9, C_I9H:C_I9H + 9] = 0.5 * np.eye(9, dtype=np.float32)
    pid = _pidx()
    for a in range(3):
        for b in range(3):
            cpack[pid[(a, b)], C_ET69 + 3 * a + b] = 1.0
    cpack[:3, C_I3:C_I3 + 3] = np.eye(3, dtype=np.float32)
    cpack[:9, C_V09] = 1.0 / 3.0
    cpack[:6, C_V06] = INV_SQRT3
    cpack[:3, C_SEL1:C_SEL1 + 3] = np.eye(3, dtype=np.float32)
    cpack[:3, C_SEL2 + 3:C_SEL2 + 6] = np.eye(3, dtype=np.float32)

    # Sh(dx,dy) = I + dx*E1 + dy*E2 + dx^2*E3 + dx*dy*E4 + dy^2*E5
    E1 = np.zeros((6, 6), np.float32)  # dx terms
    E1[0, 2] = -2.0
    E1[1, 4] = -1.0
    E1[2, 5] = -1.0
    E2 = np.zeros((6, 6), np.float32)  # dy terms
    E2[1, 2] = -1.0
    E2[3, 4] = -2.0
    E2[4, 5] = -1.0
    E3 = np.zeros((6, 6), np.float32)  # dx^2
    E3[0, 5] = 1.0
    E4 = np.zeros((6, 6), np.float32)  # dx*dy
    E4[1, 5] = 1.0
    E5 = np.zeros((6, 6), np.float32)  # dy^2
    E5[3, 5] = 1.0
    mats = [np.eye(6, dtype=np.float32), E1, E2, E3, E4, E5]
    for i, Em in enumerate(mats):
        cpack[:6, C_SHT + 6 * i:C_SHT + 6 * i + 6] = Em.T
    # svec masks: svec = [s2,s2,s,s2,s,1] = c2m*s2 + c1m*s + c0m
    cpack[:6, C_MSK + 0] = [1, 1, 0, 1, 0, 0]
    cpack[:6, C_MSK + 1] = [0, 0, 1, 0, 1, 0]
    cpack[:6, C_MSK + 2] = [0, 0, 0, 0, 0, 1]
    cpack[:9, C_IDN:C_IDN + 9] = np.eye(9, dtype=np.float32)
    cpack[:9, C_ONE:C_ONE + 9] = 1.0
    cpack[0, C_X5 + 2] = 1.0
    for p in range(3):
        cpack[4 * p, C_S3 + p] = 1.0
        cpack[0:3, C_M9 + 4 * p] = 1.0
    # index->centered-coords affine: xs = px/fx + bx, ys = py/fy + by
    fx, cx = float(K[0, 0]), float(K[0, 2])
    fy, cy = float(K[1, 1]), float(K[1, 2])
    coef = (1.0 / fx, -cx / fx - float(c0x),
            1.0 / fy, -cy / fy - float(c0y))
    return Mp, cpack, float(c0x), float(c0y), coef


def _tile128(a, ntiles):
    """[ntiles*128, F] -> [128, ntiles*F] with [p, t*F+f] = a[t*128+p, f]."""
    F = a.shape[1]
    return np.ascontiguousarray(
        a.reshape(ntiles, 128, F).transpose(1, 0, 2).reshape(128, ntiles * F)
    )


DEFAULT_K = np.array([[500.0, 0.0, 320.0], [0.0, 500.0, 240.0],
                      [0.0, 0.0, 1.0]], np.float32)


def build_nc(repeats=1, no_coll=False, no_tail=False, use_f32r=True,
             dbg_c=False, c0=None, coef=None, warm=True, f32r_tail=False):
    if c0 is None or coef is None:
        _, _, c0x_, c0y_, coef = host_constants(DEFAULT_K)
        c0 = (c0x_, c0y_)
    nc = bacc.Bacc("TRN2", target_bir_lowering=False, debug=False,
                   num_devices=NCORES)

    xn = nc.dram_tensor("xn", [128, RT * N], F32, kind="ExternalInput")
    xc = nc.dram_tensor("xc", [128, RT * N], F32, kind="ExternalInput")
    m1f = nc.dram_tensor("m1f", [128, CB * 6], F32, kind="ExternalInput")
    m2s = nc.dram_tensor("m2s", [128, RT * 6], F32, kind="ExternalInput")
    cpk = nc.dram_tensor("cpack", [9, C_TOT], F32, kind="ExternalInput")
    out_d = nc.dram_tensor("out", [6, 6] if dbg_c else [3, 3], F32, kind="ExternalOutput")

    warm_in = nc.dram_tensor("warm_in", [1, 1], U8)
    warm_out = nc.dram_tensor("warm_out", [NCORES, 1], U8, addr_space="Shared")
    tr_in = nc.dram_tensor("tr_in", [1, SH], F32)
    tr_out = nc.dram_tensor("tr_out", [NCORES, SH], F32, addr_space="Shared")
    cr_in = nc.dram_tensor("cr_in", [6, 6], F32)
    cr_out = nc.dram_tensor("cr_out", [NCORES * 6, 6], F32,
                            addr_space="Shared")
    stage = nc.dram_tensor("stage", [64], F32)
    mshuf = nc.dram_tensor("mshuf", [81], F32)

    groups = [list(range(NCORES))]

    with tile.TileContext(nc) as tc:
        with (
            tc.tile_pool(name="persist", bufs=1) as pp,
            tc.tile_pool(name="scratch", bufs=2) as sp,
            tc.tile_pool(name="ps_t", bufs=2, space="PSUM") as ps,
            tc.tile_pool(name="ps_T", bufs=2, space="PSUM") as psT,
            tc.tile_pool(name="ps_c", bufs=1, space="PSUM") as psc,
        ):
            for _rep in range(repeats):
                # ---------- P0: loads ----------
                # XN thirds across sync/scalar/gpsimd (row thresholds gate
                # the AllGather trigger); XC thirds follow on the same
                # queues; consts first on scalar (tiny, unblock casts).
                XN = pp.tile([128, RT * N], F32, tag="XN")
                XC = pp.tile([128, RT * N], F32, tag="XC")
                TN = N // 3
                m1s_s = pp.tile([128, CB * 6], F32, tag="m1f")
                nc.scalar.dma_start(m1s_s[:], m1f[:])
                m2s_s = pp.tile([128, RT * 6], F32, tag="m2s")
                nc.scalar.dma_start(m2s_s[:], m2s[:])
                cps = pp.tile([9, C_TOT], F32, tag="cpk")
                nc.scalar.dma_start(cps[:], cpk[:])
                qs = [nc.sync, nc.scalar, nc.gpsimd]
                for t in range(RT):
                    a = t * N
                    for qi, q in enumerate(qs):
                        q.dma_start(XN[:, a + qi * TN:a + (qi + 1) * TN],
                                    xn[:, a + qi * TN:a + (qi + 1) * TN])
                for t in range(RT):
                    a = t * N
                    for qi, q in enumerate([nc.sync, nc.scalar, nc.scalar]):
                        q.dma_start(XC[:, a + qi * TN:a + (qi + 1) * TN],
                                    xc[:, a + qi * TN:a + (qi + 1) * TN])
                sqwarm = sp.tile([1, 1], F32, tag="sqwarm")
                nc.scalar.activation(sqwarm[:], cps[0:1, 0:1], AF.Sqrt)

                def XNt(t):
                    return XN[:, t * N:(t + 1) * N]

                def XCt(t):
                    return XC[:, t * N:(t + 1) * N]

                # ---------- P1: row thresholds -> coll1 ----------
                r8 = pp.tile([128, RT * 8], F32, tag="r8")
                for t in range(RT):
                    nc.vector.max(out=r8[:, t * 8:t * 8 + 8], in_=XNt(t))
                trT0 = pp.tile([128, RT], F32, tag="trT0")
                nc.vector.tensor_scalar_max(
                    trT0[:],
                    r8[:].rearrange("p (t e) -> p t e", e=8)[:, :, 2], T0)
                for t in range(RT):
                    nc.gpsimd.dma_start(tr_in[0:1, t * 128:(t + 1) * 128],
                                        trT0[:, t:t + 1])

                if no_coll:
                    nc.sync.dma_start(tr_out[0:1, :], tr_in[:])
                else:
                    nc.gpsimd.collective_compute(
                        "AllGather", OP.bypass, replica_groups=groups,
                        ins=[tr_in[:]], outs=[tr_out[:]])

                # ---------- P2: col thresholds (local, exact) ----------
                c8 = pp.tile([128, RT * 8], F32, tag="c8")
                for t in range(RT):
                    nc.vector.max(out=c8[:, t * 8:t * 8 + 8], in_=XCt(t))

                # ---------- P3 (pre-coll): Z mask + candidates ----------
                WDT = F32R if use_f32r else F32
                m2r = pp.tile([128, RT * 6], WDT, tag="m2r")
                nc.vector.tensor_copy(m2r[:], m2s_s[:])
                Wr = pp.tile([128, RT * N], WDT, tag="Wr")
                for t in range(RT):
                    tcl = c8[:, t * 8 + 2:t * 8 + 3]
                    nc.vector.scalar_tensor_tensor(
                        Wr[:, t * N:(t + 1) * N], XCt(t), tcl, XCt(t),
                        OP.is_ge, OP.mult)
                # candidate indices (slots 0..2 per tile)
                ci = pp.tile([128, RT * 8], U32, tag="ci")
                for t in range(RT):
                    nc.vector.max_index(out=ci[:, t * 8:t * 8 + 8],
                                        in_max=c8[:, t * 8:t * 8 + 8],
                                        in_values=XCt(t))
                # compact candidate values z9 [128, 9]
                KG = 3
                z9 = pp.tile([128, RT * KG], F32, tag="z9")
                for t in range(RT):
                    tcl = c8[:, t * 8 + 2:t * 8 + 3]
                    nc.vector.scalar_tensor_tensor(
                        z9[:, t * KG:(t + 1) * KG], c8[:, t * 8:t * 8 + KG],
                        tcl, c8[:, t * 8:t * 8 + KG], OP.is_ge, OP.mult)
                # candidate monomial coords from indices (arithmetic):
                # r = ci; px = r & 63; py = r >> 6; xs = px*ax+bx; ys = py*ay+by
                civ = ci[:].rearrange("p (t e) -> p t e", e=8)[:, :, 0:KG]
                pxu = pp.tile([128, RT * KG], U32, tag="pxu")
                nc.vector.tensor_scalar(
                    pxu[:].rearrange("p (t e) -> p t e", e=KG), civ,
                    W - 1, None, OP.bitwise_and)
                pyu = pp.tile([128, RT * KG], U32, tag="pyu")
                nc.vector.tensor_scalar(
                    pyu[:].rearrange("p (t e) -> p t e", e=KG), civ,
                    6, None, OP.logical_shift_right)
                pxf = pp.tile([128, RT * KG], F32, tag="pxf")
                nc.vector.tensor_copy(pxf[:], pxu[:])
                pyf = pp.tile([128, RT * KG], F32, tag="pyf")
                nc.vector.tensor_copy(pyf[:], pyu[:])
                xs9 = pp.tile([128, RT * KG], F32, tag="xs9")
                nc.vector.tensor_scalar(xs9[:], pxf[:], coef[0], coef[1],
                                        OP.mult, OP.add)
                ys9 = pp.tile([128, RT * KG], F32, tag="ys9")
                nc.vector.tensor_scalar(ys9[:], pyf[:], coef[2], coef[3],
                                        OP.mult, OP.add)

                # ---------- pre-coll Z-gram: T = m2'^T Z^T ----------
                m1r = pp.tile([128, CB * 6], WDT, tag="m1r")
                nc.vector.tensor_copy(m1r[:], m1s_s[:])
                Tsb = pp.tile([6, N], F32, tag="Tsb")
                TT = pp.tile([128, CB * 6], WDT, tag="TT")
                i6 = cps[0:6, C_IDN:C_IDN + 6]
                pc0 = psc.tile([6, 6], F32, tag="pc0")
                pc1 = psc.tile([6, 6], F32, tag="pc1")
                for ch in range(6):
                    Tp = psT.tile([6, 512], F32, tag="Tp")
                    for t in range(RT):
                        c0_ = t * N + ch * 512
                        nc.tensor.matmul(
                            Tp[:], m2r[:, t * 6:(t + 1) * 6],
                            Wr[:, c0_:c0_ + 512],
                            start=(t == 0), stop=(t == RT - 1))
                    nc.scalar.activation(Tsb[:, ch * 512:(ch + 1) * 512],
                                         Tp[:], AF.Copy)
                    for jj in range(4):
                        j = ch * 4 + jj
                        pt = ps.tile([128, 6], F32, tag="tps")
                        nc.tensor.transpose(
                            pt[:], Tsb[:, j * 128:(j + 1) * 128], i6)
                        nc.scalar.activation(TT[:, j * 6:(j + 1) * 6],
                                             pt[:], AF.Copy)
                for j in range(CB):
                    pc = pc0 if j < 12 else pc1
                    nc.tensor.matmul(pc[:], m1r[:, j * 6:(j + 1) * 6],
                                     TT[:, j * 6:(j + 1) * 6],
                                     start=(j % 12 == 0), stop=(j % 12 == 11))
                CpZ = pp.tile([6, 6], F32, tag="CpZ")
                nc.vector.tensor_copy(CpZ[:], pc0[:])
                nc.vector.tensor_tensor(CpZ[:], CpZ[:], pc1[:], OP.add)

                # ---------- post-coll: sparse row-test correction ----------
                # g8[p, t*3+k] = trT0full[ci[p, t*8+k]] -- 9 gathers
                g8 = pp.tile([128, RT * KG], F32, tag="g8")
                trflat = tr_out[:].rearrange("k i -> (k i)").unsqueeze(1)
                for t in range(RT):
                    for k in range(KG):
                        sl = t * KG + k
                        nc.gpsimd.indirect_dma_start(
                            out=g8[:, sl:sl + 1], out_offset=None,
                            in_=trflat,
                            in_offset=bass.IndirectOffsetOnAxis(
                                ap=ci[:, t * 8 + k:t * 8 + k + 1], axis=0))
                # fused correction: prod strips [e1x, e1y, e1, e2y, e2, d]
                # per tile (pipelines behind that tile's three gathers)
                ccp = ps.tile([6, 6], F32, tag="tps")
                TTc = sp.tile([128, RT * 6], WDT, tag="TTc")
                for t in range(RT):
                    tsl = slice(t * KG, (t + 1) * KG)
                    prod = sp.tile([128, 6 * KG], F32, tag=f"prod{t}")

                    def strip(c):
                        return prod[:, c * KG:(c + 1) * KG]
                    d_ = strip(5)
                    nc.vector.tensor_tensor(d_, z9[:, tsl], g8[:, tsl],
                                            OP.is_lt)
                    nc.vector.tensor_tensor(d_, d_, z9[:, tsl], OP.mult)
                    nc.vector.tensor_tensor(strip(2), d_, xs9[:, tsl], OP.mult)
                    nc.vector.tensor_tensor(strip(4), d_, ys9[:, tsl], OP.mult)
                    nc.vector.tensor_tensor(strip(0), strip(2), xs9[:, tsl],
                                            OP.mult)
                    nc.vector.tensor_tensor(strip(1), strip(2), ys9[:, tsl],
                                            OP.mult)
                    nc.vector.tensor_tensor(strip(3), strip(4), ys9[:, tsl],
                                            OP.mult)
                    with nc.allow_low_precision(reason="f32r is f32-width"):
                        nc.vector.tensor_reduce(
                            TTc[:, t * 6:(t + 1) * 6].unsqueeze(2),
                            prod[:].rearrange("p (c s) -> p c s", s=KG),
                            AX.X, OP.add)
                    nc.tensor.matmul(ccp[:], TTc[:, t * 6:(t + 1) * 6],
                                     m2r[:, t * 6:(t + 1) * 6],
                                     start=(t == 0), stop=(t == RT - 1))
                Cp = sp.tile([6, 6], F32, tag="Cp")
                nc.vector.tensor_tensor(Cp[:], CpZ[:], ccp[:], OP.subtract)
                nc.gpsimd.dma_start(cr_in[:], Cp[:])

                # ---------- coll2: AllGather 6x6 grams, sum locally ----------
                if no_coll:
                    zz = sp.tile([(NCORES - 1) * 6, 6], F32, tag="zz")
                    nc.vector.memset(zz[:], 0.0)
                    nc.sync.dma_start(cr_out[0:6, :], cr_in[:])
                    nc.sync.dma_start(cr_out[6:NCORES * 6, :], zz[:])
                else:
                    nc.gpsimd.collective_compute(
                        "AllGather", OP.bypass, replica_groups=groups,
                        ins=[cr_in[:]], outs=[cr_out[:]])

                if no_tail:
                    nn = 6 if dbg_c else 3
                    dummy = sp.tile([nn, nn], F32, tag="dummy")
                    nc.sync.dma_start(dummy[:], cr_out[0:nn, 0:nn])
                    nc.sync.dma_start(out_d[:], dummy[:])
                    continue

                # ---------- tail ----------
                _tail(nc, pp, sp, ps, psc, cps, c0, cr_out, stage, mshuf,
                      out_d, f32r_tail)

    nc.compile()
    return nc


def _transpose(nc, ps, sp, in_sb, n, idn, tag, dt=F32):
    pt = ps.tile([n, n], F32, tag="tps")
    nc.tensor.transpose(pt[:], in_sb, idn[:n, :n])
    ot = sp.tile([n, n], dt, tag=f"ot_{tag}")
    nc.vector.tensor_copy(ot[:], pt[:])
    return ot


def _powchain(nc, ps, sp, m_sb, n, tag, n_squarings=5, extra=True, dt=F32):
    """M^50 (extra=True: 5 squarings + M48=M32@M16 + M50=M48@M2) or M^32.

    Intermediates use dt (f32r halves instruction count); the returned
    final power is always F32 so downstream vector-extract matmuls stay
    within fp32r ISA restrictions.
    """
    powers = {}
    cur = m_sb
    for i in range(1, n_squarings + 1):
        last = (i == n_squarings) and not extra
        pm = ps.tile([n, n], F32, tag="tps")
        nc.tensor.matmul(pm[:], cur, cur, start=True, stop=True)
        nxt = sp.tile([n, n], F32 if last else dt, tag=f"pw_{tag}_{i}")
        nc.vector.tensor_scalar_mul(nxt[:], pm[:], 2.0)
        powers[2 ** i] = nxt
        cur = nxt[:]
    if not extra:
        return powers[2 ** n_squarings]
    # M50 = M32 @ (M16 @ M2): the M18 matmul is independent of the M32
    # squaring, so it overlaps instead of serializing after it
    pm = ps.tile([n, n], F32, tag="tps")
    nc.tensor.matmul(pm[:], powers[16][:], powers[2][:], start=True, stop=True)
    m18 = sp.tile([n, n], dt, tag=f"pw_{tag}_18")
    nc.vector.tensor_scalar_mul(m18[:], pm[:], 2.0)
    pm = ps.tile([n, n], F32, tag="tps")
    nc.tensor.matmul(pm[:], powers[32][:], m18[:], start=True, stop=True)
    m50 = sp.tile([n, n], F32, tag=f"pw_{tag}_50")
    nc.vector.tensor_scalar_mul(m50[:], pm[:], 2.0)
    return m50


def _tail(nc, pp, sp, ps, psc, cps, c0, cr_out, stage, mshuf, out_d,
          f32r_tail):
    """C' -> Hartley -> L-transform -> Mmat -> chains -> projection."""
    idn = cps[0:9, C_IDN:C_IDN + 9]

    # read gathered grams, sum over cores: CpBoth = [C'^T | C']
    csum = sp.tile([6, NCORES * 6], F32, tag="csum")
    nc.sync.dma_start(
        csum[:].rearrange("r (k c) -> r k c", c=6),
        cr_out[:].rearrange("(k r) c -> r k c", r=6))
    CpBoth = sp.tile([6, 12], F32, tag="CpBoth")
    Cp = CpBoth[:, 6:12]
    nc.vector.tensor_reduce(
        Cp.unsqueeze(2),
        csum[:].rearrange("r (k c) -> r c k", c=6), AX.X, OP.add)
    ptC = ps.tile([6, 6], F32, tag="tps")
    nc.tensor.transpose(ptC[:], Cp, cps[0:6, C_IDN:C_IDN + 6])
    nc.vector.tensor_copy(CpBoth[:, 0:6], ptC[:])

    # moments [1,12] via PE row-extract: sc[0:6]=row5(C'^T), sc[6:12]=row5(C')
    sc = pp.tile([128, 112], F32, tag="tailsc")
    e5 = cps[0:6, C_IDN + 5:C_IDN + 6]
    scm = ps.tile([1, 12], F32, tag="tps")
    nc.tensor.matmul(scm[:], e5, CpBoth[:], start=True, stop=True)
    nc.vector.tensor_copy(sc[0:1, 0:12], scm[:])

    def scv(a, b):
        return sc[0:1, a:b]

    def pair(k):
        return sc[0:1, 0:12].rearrange("p (g d) -> p d g", g=2)[:, k, :]

    Sxx, Sx, Syy, Sy, Sw = pair(0), pair(2), pair(3), pair(4), pair(5)
    ws = scv(12, 14); nc.vector.tensor_scalar_add(ws, Sw, EPS)
    rws = scv(14, 16); nc.vector.reciprocal(rws, ws)
    cx = scv(16, 18); nc.vector.tensor_tensor(cx, Sx, rws, OP.mult)  # = dx
    cy = scv(18, 20); nc.vector.tensor_tensor(cy, Sy, rws, OP.mult)  # = dy
    t_a = scv(20, 22); nc.vector.tensor_tensor(t_a, cx, Sx, OP.mult)
    t_b = scv(22, 24); nc.vector.tensor_tensor(t_b, cy, Sy, OP.mult)
    cdS = scv(24, 26); nc.vector.tensor_tensor(cdS, t_a, t_b, OP.add)
    u_a = scv(26, 28); nc.vector.tensor_tensor(u_a, cx, cx, OP.mult)
    u_b = scv(28, 30); nc.vector.tensor_tensor(u_b, cy, cy, OP.mult)
    c2_ = scv(30, 32); nc.vector.tensor_tensor(c2_, u_a, u_b, OP.add)
    sq_ = scv(32, 34); nc.vector.tensor_tensor(sq_, Sxx, Syy, OP.add)
    n2c = scv(34, 36); nc.vector.tensor_scalar_mul(n2c, cdS, -2.0)
    c2w = scv(36, 38); nc.vector.tensor_tensor(c2w, c2_, Sw, OP.mult)
    m_ = scv(38, 40); nc.vector.tensor_tensor(m_, sq_, n2c, OP.add)
    m2_ = scv(40, 42); nc.vector.tensor_tensor(m2_, m_, c2w, OP.add)
    md2 = scv(42, 44); nc.vector.tensor_tensor(md2, m2_, rws, OP.mult)
    md2e = scv(44, 46); nc.vector.tensor_scalar_add(md2e, md2, EPS)
    md = scv(46, 48); nc.scalar.activation(md, md2e, AF.Sqrt)
    mde = scv(48, 50); nc.vector.tensor_scalar_add(mde, md, EPS)
    rmd = scv(50, 52); nc.vector.reciprocal(rmd, mde)
    s_ = scv(52, 54); nc.vector.tensor_scalar_mul(s_, rmd, SQRT2)
    # real centroids: cr = dx + c0 ; then -s*cr
    cxr = scv(54, 56); nc.vector.tensor_scalar_add(cxr, cx, c0[0])
    cyr = scv(56, 58); nc.vector.tensor_scalar_add(cyr, cy, c0[1])
    scx = scv(58, 60); nc.vector.tensor_tensor(scx, s_, cxr, OP.mult)
    scy = scv(60, 62); nc.vector.tensor_tensor(scy, s_, cyr, OP.mult)
    nscx = scv(62, 64); nc.vector.tensor_scalar_mul(nscx, scx, -1.0)
    nscy = scv(64, 66); nc.vector.tensor_scalar_mul(nscy, scy, -1.0)
    # L scalars: s2, dx2, dxy, dy2 (paired)
    s2p = scv(66, 68); nc.vector.tensor_tensor(s2p, s_, s_, OP.mult)
    dx2 = scv(68, 70); nc.vector.tensor_tensor(dx2, cx, cx, OP.mult)
    dxy = scv(70, 72); nc.vector.tensor_tensor(dxy, cx, cy, OP.mult)
    dy2 = scv(72, 74); nc.vector.tensor_tensor(dy2, cy, cy, OP.mult)

    # broadcast scalar strip to 6 partitions (PE ones); consumers read PSUM
    ones16 = cps[0:1, C_ONE:C_ONE + 6]
    scBt = psc.tile([6, 80], F32, tag="scBp")
    nc.tensor.matmul(scBt[:], ones16, sc[0:1, 0:80], start=True, stop=True)
    scB = scBt

    # T row-major 9-vectors: t1v at 76:85, t2v at 85:94 (stage bounce --
    # partition-offset writes are illegal on compute engines)
    nc.vector.memset(scv(76, 94), 0.0)
    tv = sc[0:1, 76:94]
    tv9 = tv.rearrange("p (v f) -> p v f", v=2)
    nc.vector.tensor_copy(tv9[:, :, 0:1], s_.unsqueeze(2))
    nc.vector.tensor_copy(tv9[:, :, 4:5], s_.unsqueeze(2))
    nc.vector.tensor_copy(
        tv9[:, :, 2:8].rearrange("p v (c d) -> p v c d", c=2)[:, :, :, 0:1],
        sc[0:1, 62:66].rearrange("p (c v) -> p v c", c=2).unsqueeze(3))
    nc.vector.memset(tv9[:, :, 8:9], 1.0)
    # gpsimd queue: keeps the sync DMA lane free for the Mmat bounce
    nc.gpsimd.dma_start(stage[0:18], tv)
    T12 = sp.tile([3, 6], F32, tag="T12")
    nc.gpsimd.dma_start(
        T12[:].rearrange("i (v j) -> i v j", v=2),
        stage[0:18].rearrange("(v i j) -> i v j", i=3, j=3))

    def shT(side, tag, eng, srcB):
        """Sh^T for side (0/1): I^T + dx E1^T + dy E2^T + dx2 E3^T + ..."""
        dx = srcB[:, 16 + side:17 + side]
        dy = srcB[:, 18 + side:19 + side]
        dx2_ = srcB[:, 68 + side:69 + side]
        dxy_ = srcB[:, 70 + side:71 + side]
        dy2_ = srcB[:, 72 + side:73 + side]
        def M(i):
            return cps[0:6, C_SHT + 6 * i:C_SHT + 6 * i + 6]
        acc = sp.tile([6, 6], F32, tag=f"sh_{tag}")
        eng.scalar_tensor_tensor(acc[:], M(1), dx, M(0), OP.mult, OP.add)
        for i, sval in [(2, dy), (3, dx2_), (4, dxy_), (5, dy2_)]:
            eng.scalar_tensor_tensor(acc[:], M(i), sval, acc[:],
                                     OP.mult, OP.add)
        return acc

    Sh1T = shT(0, "1", nc.vector, scB)
    Sh2T = shT(1, "2", nc.vector, scB)
    # svec side1 as a [6,1] column (per-partition): c2m*s2 + c1m*s + c0m
    sv1c = sp.tile([6, 1], F32, tag="sv1c")
    tmp1 = sp.tile([6, 1], F32, tag="svt1")
    nc.vector.scalar_tensor_tensor(
        tmp1[:], cps[0:6, C_MSK:C_MSK + 1], scB[:, 66:67],
        cps[0:6, C_MSK + 2:C_MSK + 3], OP.mult, OP.add)
    nc.vector.scalar_tensor_tensor(
        sv1c[:], cps[0:6, C_MSK + 1:C_MSK + 2], scB[:, 52:53],
        tmp1[:], OP.mult, OP.add)
    # svec side2 as a [1,6] row on partition 0: [s2 s2 s s2 s 1]
    svr2 = sc[0:1, 96:102]
    s2v2 = sc[0:1, 67:68]
    sv2 = sc[0:1, 53:54]
    nc.vector.tensor_copy(
        svr2.rearrange("p (a b) -> p a b", a=3)[:, 0:2, 0:1],
        s2v2.unsqueeze(2).to_broadcast([1, 2, 1]))   # slots 0,2 = s2 (a-major)
    nc.vector.tensor_copy(svr2[:, 1:2], s2v2)        # slot 1 = s2
    nc.vector.tensor_copy(svr2[:, 3:4], s2v2)        # slot 3 = s2
    nc.vector.tensor_copy(svr2[:, 2:3], sv2)         # slot 2 = s
    nc.vector.tensor_copy(svr2[:, 4:5], sv2)         # slot 4 = s
    nc.vector.memset(svr2[:, 5:6], 1.0)
    sv2B = sp.tile([6, 6], F32, tag="sv2B")
    sv2Bp = ps.tile([6, 6], F32, tag="tps")
    nc.tensor.matmul(sv2Bp[:], ones16, svr2, start=True, stop=True)
    nc.vector.tensor_copy(sv2B[:], sv2Bp[:])

    # C2 = D1 Sh1 C' Sh2^T D2
    vps = ps.tile([6, 6], F32, tag="tps")
    nc.tensor.matmul(vps[:], Sh1T[:], Cp, start=True, stop=True)  # Sh1 C'
    vS = sp.tile([6, 6], F32, tag="vS")
    nc.vector.tensor_copy(vS[:], vps[:])
    vT = _transpose(nc, ps, sp, vS[:], 6, idn, "vT")
    ups = ps.tile([6, 6], F32, tag="tps")
    nc.tensor.matmul(ups[:], vT[:], Sh2T[:], start=True, stop=True)  # v Sh2^T
    # C2[r, c] = svec1[r] * u[r, c] * svec2[c]
    u1 = sp.tile([6, 6], F32, tag="u1")
    nc.vector.tensor_scalar_mul(u1[:], ups[:], sv1c[:])
    C2 = sp.tile([6, 6], F32, tag="C2")
    nc.vector.tensor_tensor(C2[:], u1[:], sv2B[:], OP.mult)
    C2T = _transpose(nc, ps, sp, C2[:], 6, idn, "c2t")

    _solve(nc, pp, sp, ps, psc, cps, idn, sc, C2[:], C2T[:], stage, mshuf,
           out_d, T12, f32r_tail)


def _solve(nc, pp, sp, ps, psc, cps, idn, sc, C2, C2T, stage, mshuf, out_d,
           T12, f32r_tail):
    PDT = F32R if f32r_tail else F32
    i9h = cps[0:9, C_I9H:C_I9H + 9]
    et69 = cps[0:6, C_ET69:C_ET69 + 9]
    i3c = cps[0:3, C_I3:C_I3 + 3]
    v09 = cps[0:9, C_V09:C_V09 + 1]
    v06 = cps[0:6, C_V06:C_V06 + 1]
    sel1 = cps[0:3, C_SEL1:C_SEL1 + 6]
    sel2 = cps[0:3, C_SEL2:C_SEL2 + 6]

    # G2 = E C2 E^T : G2[3a+b, 3c+d] = C2[pair(a,b), pair(c,d)]
    z_ps = ps.tile([6, 9], F32, tag="tps")
    nc.tensor.matmul(z_ps[:], C2T, et69, start=True, stop=True)  # C2 E^T
    Zs = sp.tile([6, 9], F32, tag="Zs")
    nc.vector.tensor_copy(Zs[:], z_ps[:])
    g_ps = ps.tile([9, 9], F32, tag="tps")
    nc.tensor.matmul(g_ps[:], et69, Zs[:], start=True, stop=True)    # E @ Z
    G2 = sp.tile([9, 9], F32, tag="G2")
    nc.vector.tensor_copy(G2[:], g_ps[:])

    # Mmat[3p+q, 3r+s] = G2[3p+r, 3q+s]: bounce via DRAM, split per
    # 3-row block so each read only waits its own write's receipt
    Mmat = sp.tile([9, 9], F32, tag="Mmat")
    for p in range(3):
        eng = nc.scalar if p == 1 else nc.sync
        eng.dma_start(mshuf[27 * p:27 * p + 27], G2[3 * p:3 * p + 3, :])
        eng.dma_start(
            Mmat[3 * p:3 * p + 3, :].rearrange("q (r s) -> q r s", s=3),
            mshuf[:].rearrange("(p q1 r s) -> p q1 r s", p=3, q1=3, r=3)
            .transpose([0, 2, 1, 3])[p])

    # lam = trace(Mmat) = sum G2[{0,4,8},{0,4,8}] -- from G2, overlapping
    # the Mmat DRAM bounce
    s3 = cps[0:9, C_S3:C_S3 + 3]
    d3ps = ps.tile([3, 9], F32, tag="tps")
    nc.tensor.matmul(d3ps[:], s3, G2[:], start=True, stop=True)
    d3 = sp.tile([3, 9], F32, tag="d3")
    nc.vector.tensor_tensor(d3[:], d3ps[:], cps[0:3, C_M9:C_M9 + 9], OP.mult)
    lam3 = sp.tile([3, 1], F32, tag="lam3")
    nc.vector.tensor_reduce(lam3[:], d3[:], AX.X, OP.add)
    lam2r = ps.tile([9, 1], F32, tag="tps")
    nc.tensor.matmul(lam2r[:], cps[0:3, C_ONE:C_ONE + 9], lam3[:],
                     start=True, stop=True)
    lam4 = sp.tile([9, 1], F32, tag="lam4")
    nc.vector.tensor_scalar_mul(lam4[:], lam2r[:], 2.0)
    inv2l = sp.tile([9, 1], F32, tag="inv2l")
    nc.vector.reciprocal(inv2l[:], lam4[:])
    # fp32r matmul needs even stationary-free: run the 9x9 chain as 10x10
    ND = 10 if f32r_tail else 9
    Msp = sp.tile([ND, ND], PDT, tag="Msp")
    if ND != 9:
        nc.vector.memset(Msp[:].bitcast(F32), 0.0)
    nc.vector.scalar_tensor_tensor(Msp[0:9, 0:9], Mmat[:], inv2l[:], i9h,
                                   OP.mult, OP.subtract)
    M50 = _powchain(nc, ps, sp, Msp[:], ND, "m9", 5, extra=True, dt=PDT)

    v09p = sp.tile([ND, 1], F32, tag="v09p")
    if ND != 9:
        nc.vector.memset(v09p[:], 0.0)
    nc.vector.tensor_copy(v09p[0:9, :], v09)
    w9ps = ps.tile([1, ND], F32, tag="tps")
    nc.tensor.matmul(w9ps[:], v09p[:], M50[:], start=True, stop=True)
    w9 = sp.tile([1, 9], F32, tag="w9")
    nc.vector.tensor_copy(w9[:], w9ps[0:1, 0:9])
    w9sq = sp.tile([1, 9], F32, tag="w9sq")
    nc.vector.tensor_tensor(w9sq[:], w9[:], w9[:], OP.mult)
    nn9 = sp.tile([1, 1], F32, tag="nn9")
    nc.vector.tensor_reduce(nn9[:], w9sq[:], AX.X, OP.add)
    sr9 = sp.tile([1, 1], F32, tag="sr9")
    nc.scalar.activation(sr9[:], nn9[:], AF.Sqrt)
    rs9 = sp.tile([1, 1], F32, tag="rs9")
    nc.vector.reciprocal(rs9[:], sr9[:])
    rs9c = psc.tile([3, 1], F32, tag="rs9c")
    nc.tensor.matmul(rs9c[:], cps[0:1, C_ONE:C_ONE + 3], rs9[:],
                     start=True, stop=True)

    # E = T2^T E_raw T1 (and E^T);  T1m/T2m preloaded in T12
    # Eraw [3,3] from w9 [1,9] via rank-1 sums: sum_b e_b (x) w9[3b:3b+3]
    # (raw; 1/||w9|| folded at the end)
    T1m = T12[:, 0:3]
    T2m = T12[:, 3:6]
    erps = ps.tile([3, 3], F32, tag="tps")
    for b in range(3):
        nc.tensor.matmul(erps[:], cps[0:1, C_X5 + 2 - b:C_X5 + 5 - b],
                         w9[0:1, 3 * b:3 * b + 3],
                         start=(b == 0), stop=(b == 2))
    Eraw = sp.tile([3, 3], F32, tag="Eraw")
    nc.vector.tensor_copy(Eraw[:], erps[:])

    a1ps = ps.tile([3, 3], F32, tag="tps")
    nc.tensor.matmul(a1ps[:], T2m, Eraw[:], start=True, stop=True)
    A1 = sp.tile([3, 3], F32, tag="A1")
    nc.vector.tensor_copy(A1[:], a1ps[:])
    A1T = _transpose(nc, ps, sp, A1[:], 3, idn, "a1t")
    etps = ps.tile([3, 3], F32, tag="tps")
    nc.tensor.matmul(etps[:], T1m, A1T[:], start=True, stop=True)
    ETs = sp.tile([3, 3], F32, tag="ETs")
    nc.vector.tensor_copy(ETs[:], etps[:])
    Es = _transpose(nc, ps, sp, ETs[:], 3, idn, "es")

    # B = E^T E ; blockdiag 6x6 chain (32 iters) for v1 (max) and v3 (min)
    bps = ps.tile([3, 3], F32, tag="tps")
    nc.tensor.matmul(bps[:], Es[:], Es[:], start=True, stop=True)
    Bm = sp.tile([3, 3], F32, tag="Bm")
    nc.vector.tensor_copy(Bm[:], bps[:])
    dg3 = sp.tile([3, 3], F32, tag="dg3")
    nc.vector.tensor_tensor(dg3[:], Bm[:], i3c, OP.mult)
    lb = sp.tile([3, 1], F32, tag="lb")
    nc.vector.tensor_reduce(lb[:], dg3[:], AX.X, OP.add)
    lbr = ps.tile([3, 1], F32, tag="tps")
    nc.tensor.matmul(lbr[:], cps[0:3, C_ONE:C_ONE + 3], lb[:],
                     start=True, stop=True)
    invlb = sp.tile([3, 1], F32, tag="invlb")
    nc.vector.reciprocal(invlb[:], lbr[:])
    Bs3 = sp.tile([3, 3], F32, tag="Bs3")
    nc.vector.tensor_scalar_mul(Bs3[:], Bm[:], invlb[:])
    IB = sp.tile([3, 3], F32, tag="IB")
    nc.vector.tensor_tensor(IB[:], i3c, Bs3[:], OP.subtract)
    bdps = ps.tile([6, 6], F32, tag="tps")
    nc.tensor.matmul(bdps[:, 0:3], sel1, Bs3[:], start=True, stop=True)
    nc.tensor.matmul(bdps[:, 3:6], sel2, IB[:], start=True, stop=True)
    BD = sp.tile([6, 6], PDT, tag="BD")
    nc.vector.tensor_copy(BD[:], bdps[:])
    BD32 = _powchain(nc, ps, sp, BD[:], 6, "m6", 5, extra=False, dt=PDT)

    w6ps = ps.tile([1, 6], F32, tag="tps")
    nc.tensor.matmul(w6ps[:], v06, BD32[:], start=True, stop=True)
    w6 = sp.tile([1, 6], F32, tag="w6")
    nc.vector.tensor_copy(w6[:], w6ps[:])
    w6sq = sp.tile([1, 6], F32, tag="w6sq")
    nc.vector.tensor_tensor(w6sq[:], w6[:], w6[:], OP.mult)
    nn6 = sp.tile([1, 2], F32, tag="nn6")
    nc.vector.tensor_reduce(nn6[:].unsqueeze(2),
                            w6sq[:].rearrange("p (g d) -> p g d", g=2), AX.X,
                            OP.add)
    sr6 = sp.tile([1, 2], F32, tag="sr6")
    nc.scalar.activation(sr6[:], nn6[:], AF.Sqrt)
    rs6 = sp.tile([1, 2], F32, tag="rs6")
    nc.vector.reciprocal(rs6[:], sr6[:])
    vv = sp.tile([1, 6], F32, tag="vv")
    nc.vector.tensor_tensor(
        vv[:].rearrange("p (g d) -> p g d", g=2),
        w6[:].rearrange("p (g d) -> p g d", g=2),
        rs6[:].unsqueeze(2).to_broadcast([1, 2, 3]), OP.mult)

    # v2 = cross(v3, v1), normalized with EPS
    aa = sp.tile([1, 6], F32, tag="aa")
    nc.vector.tensor_copy(
        aa[:].rearrange("p (r d) -> p r d", r=2),
        vv[:, 3:6].unsqueeze(1).to_broadcast([1, 2, 3]))
    bb = sp.tile([1, 6], F32, tag="bb")
    nc.vector.tensor_copy(
        bb[:].rearrange("p (r d) -> p r d", r=2),
        vv[:, 0:3].unsqueeze(1).to_broadcast([1, 2, 3]))
    cr1 = sp.tile([1, 3], F32, tag="cr1")
    nc.vector.tensor_tensor(cr1[:], aa[:, 1:4], bb[:, 2:5], OP.mult)
    cr2 = sp.tile([1, 3], F32, tag="cr2")
    nc.vector.tensor_tensor(cr2[:], aa[:, 2:5], bb[:, 1:4], OP.mult)
    v2r = sp.tile([1, 3], F32, tag="v2r")
    nc.vector.tensor_tensor(v2r[:], cr1[:], cr2[:], OP.subtract)
    v2sq = sp.tile([1, 3], F32, tag="v2sq")
    nc.vector.tensor_tensor(v2sq[:], v2r[:], v2r[:], OP.mult)
    nn2 = sp.tile([1, 1], F32, tag="nn2")
    nc.vector.tensor_reduce(nn2[:], v2sq[:], AX.X, OP.add)
    sr2 = sp.tile([1, 1], F32, tag="sr2")
    nc.scalar.activation(sr2[:], nn2[:], AF.Sqrt)
    sr2e = sp.tile([1, 1], F32, tag="sr2e")
    nc.vector.tensor_scalar_add(sr2e[:], sr2[:], EPS)
    rs2 = sp.tile([1, 1], F32, tag="rs2")
    nc.vector.reciprocal(rs2[:], sr2e[:])
    v2 = sp.tile([1, 3], F32, tag="v2")
    nc.vector.tensor_tensor(v2[:], v2r[:], rs2[:].to_broadcast([1, 3]), OP.mult)

    vvv = sp.tile([1, 6], F32, tag="vvv")
    nc.vector.tensor_copy(vvv[:, 0:3], vv[:, 0:3])
    nc.vector.tensor_copy(vvv[:, 3:6], v2[:])
    # Vr [2,3] rows from vvv halves; Vc [3,2] = Vr^T -- both via rank-1 MMs
    vrps = ps.tile([2, 3], F32, tag="tps")
    for r in range(2):
        nc.tensor.matmul(vrps[:], cps[0:1, C_X5 + 2 - r:C_X5 + 4 - r],
                         vvv[0:1, 3 * r:3 * r + 3],
                         start=(r == 0), stop=(r == 1))
    Vr = sp.tile([2, 3], F32, tag="Vr")
    nc.vector.tensor_copy(Vr[:], vrps[:])
    vcps = ps.tile([3, 2], F32, tag="tps")
    for r in range(2):
        nc.tensor.matmul(vcps[:], vvv[0:1, 3 * r:3 * r + 3],
                         cps[0:1, C_X5 + 2 - r:C_X5 + 4 - r],
                         start=(r == 0), stop=(r == 1))
    Vc = sp.tile([3, 2], F32, tag="Vc")
    nc.vector.tensor_copy(Vc[:], vcps[:])
    evps = ps.tile([2, 3], F32, tag="tps")
    nc.tensor.matmul(evps[:], Vc[:], ETs[:], start=True, stop=True)
    Evr = sp.tile([2, 3], F32, tag="Evr")
    nc.vector.tensor_copy(Evr[:], evps[:])
    evsq = sp.tile([2, 3], F32, tag="evsq")
    nc.vector.tensor_tensor(evsq[:], Evr[:], Evr[:], OP.mult)
    ss2 = sp.tile([2, 1], F32, tag="ss2")
    nc.vector.tensor_reduce(ss2[:], evsq[:], AX.X, OP.add)
    sv = sp.tile([2, 1], F32, tag="sv")
    nc.scalar.activation(sv[:], ss2[:], AF.Sqrt)
    ssum = ps.tile([2, 1], F32, tag="tps")
    nc.tensor.matmul(ssum[:], cps[0:2, C_ONE:C_ONE + 2], sv[:],
                     start=True, stop=True)
    savg = sp.tile([2, 1], F32, tag="savg")
    nc.vector.tensor_scalar_mul(savg[:], ssum[:], 0.5)
    sve = sp.tile([2, 1], F32, tag="sve")
    nc.vector.tensor_scalar_add(sve[:], sv[:], EPS)
    rsv = sp.tile([2, 1], F32, tag="rsv")
    nc.vector.reciprocal(rsv[:], sve[:])
    f2 = sp.tile([2, 1], F32, tag="f2")
    nc.vector.tensor_tensor(f2[:], rsv[:], savg[:], OP.mult)
    U2 = sp.tile([2, 3], F32, tag="U2")
    nc.vector.tensor_scalar_mul(U2[:], Evr[:], f2[:])
    ops_ = ps.tile([3, 3], F32, tag="tps")
    nc.tensor.matmul(ops_[:], U2[:], Vr[:], start=True, stop=True)
    outs = sp.tile([3, 3], F32, tag="outs")
    nc.vector.tensor_scalar_mul(outs[:], ops_[:], rs9c[:])
    nc.sync.dma_start(out_d[:], outs[:])


def make_in_maps(P, K):
    P = np.asarray(P, np.float32)
    K = np.asarray(K, np.float32)
    Pc = np.ascontiguousarray(P[:N, :N])
    PcT = np.ascontiguousarray(Pc.T)
    Mp, cpack, c0x, c0y, coef = host_constants(K)
    m1full = _tile128(Mp, CB)
    in_maps = []
    for k in range(NCORES):
        in_maps.append({
            "xn": _tile128(Pc[k * SH:(k + 1) * SH], RT),
            "xc": _tile128(PcT[k * SH:(k + 1) * SH], RT),
            "m1f": m1full,
            "m2s": _tile128(Mp[k * SH:(k + 1) * SH], RT),
            "cpack": cpack,
        })
    return in_maps


_NC_CACHE = {}


def kernel(P, K):
    from concourse.bass_utils import run_bass_kernel_spmd
    if "nc" not in _NC_CACHE:
        _, _, c0x, c0y, coef = host_constants(np.asarray(K, np.float32))
        _NC_CACHE["nc"] = build_nc(c0=(c0x, c0y), coef=coef)
    nc = _NC_CACHE["nc"]
    in_maps = make_in_maps(P, K)
    res = run_bass_kernel_spmd(nc, in_maps, core_ids=list(range(NCORES)))
    return np.asarray(res.results[0]["out"], np.float32)
